# revision 12
# baseline (speedup 1.0000x reference)
"""Multi-head attention (EMB=512, HEADS=8, x:(4,2048,512)) on 8 Trainium2 cores.

Sharding: zero-collective split — core c handles batch c//2, query rows
(c%2)*1024..(c%2+1)*1024, ALL heads.  K/V projections for the full batch are
computed redundantly on the 2 cores sharing a batch (no collectives at all).

All matmul operands are bf16: on TRN2 a 128-deep-contraction fp32r matmul
runs 2 cycles/row ("HIGH" replicated mode) while bf16 runs 1 cycle/row with
FWL weight loads, so bf16 halves projection time and halves input DMA bytes.

Device-side dataflow (per core, everything SBUF-resident):
  xT (host-transposed)           [512, 2048]   keys reordered so queries first
  Q^T = WqT.T @ xT  (+bq)        [512, 1024]   feature-major (bf16)
  K^T = WkT.T @ xT  (+bk)        [512, 2048]   feature-major (bf16)
  V~  = xT.T @ WvT  (+bv, ones)  [2048, 8*72]  token-major, per-head ones col
  S^T(t) = K^T_h.T @ Q^T_h       one [128,1024] PSUM tile per key-tile t holds
                                 BOTH heads of the pair: cols 0:512 head A on
                                 PE rows 0:64, cols 512:1024 head B on rows
                                 64:128 — emitted adjacently so the pair runs
                                 concurrently via PE row tiling
  P^T = exp(S^T / sqrt(512))     (ScalarE, fused drain from PSUM -> bf16)
  outT~ = V~_h.T @ P^T           [72, 512]     row 64+h = softmax denominator
  outT = outT~ * bcast(1/denom)  normalization fused into the PSUM drain
  y = outT.T @ WoT (+bo)         [1024, 512]   token-major, DMA out

Loop order: query-chunk c is OUTER so chunk 0's output projection + DMA
overlap chunk 1's (ScalarE-bound) attention.  Startup is kt-outer so the
first Q/K projections overlap the input DMA stream.  V-projection tiles are
emitted inside the first attention block's t-loop, just in time for PV.
"""

import sys
import os

for _p in ("/opt/trn_rl_repo", "/root/.axon_site/_ro/trn_rl_repo"):
    if os.path.isdir(_p) and _p not in sys.path:
        sys.path.append(_p)

import numpy as np

EMB = 512
HEADS = 8
D = 64  # head dim
B = 4
N = 2048  # keys / tokens per batch
HALF = 1024  # queries per core
P = 128
NCORES = 8
KT4 = EMB // P  # 4 contraction tiles
SCALE = float(1.0 / np.sqrt(np.float32(EMB)))

_CACHE = {}


def _build_program(debug=False):
    from concourse import bacc
    import concourse.mybir as mybir
    import concourse.tile as tile
    from contextlib import ExitStack

    dt = mybir.dt.float32
    bf16 = mybir.dt.bfloat16
    nc = bacc.Bacc("TRN2", target_bir_lowering=False)

    xT_d = nc.dram_tensor("xT", [KT4, P, N], bf16, kind="ExternalInput")
    wq_d = nc.dram_tensor("wq", [KT4, P, EMB], bf16, kind="ExternalInput")
    wk_d = nc.dram_tensor("wk", [KT4, P, EMB], bf16, kind="ExternalInput")
    wv_d = nc.dram_tensor("wv", [KT4, P, EMB], bf16, kind="ExternalInput")
    wo_d = nc.dram_tensor("wo", [KT4, P, EMB], bf16, kind="ExternalInput")
    bq_d = nc.dram_tensor("bq2", [P, KT4], dt, kind="ExternalInput")
    bk_d = nc.dram_tensor("bk2", [P, KT4], dt, kind="ExternalInput")
    bvr_d = nc.dram_tensor("bvr", [P, EMB], dt, kind="ExternalInput")
    bor_d = nc.dram_tensor("bor", [P, EMB], dt, kind="ExternalInput")
    sel_d = nc.dram_tensor("sel2", [2, P], bf16, kind="ExternalInput")  # row 0: p<64, row 1: p>=64
    y_d = nc.dram_tensor("y", [HALF, EMB], dt, kind="ExternalOutput")
    if debug:
        dQT = nc.dram_tensor("dQT", [P, KT4, HALF], bf16, kind="ExternalOutput")
        dKT = nc.dram_tensor("dKT", [P, KT4, N], bf16, kind="ExternalOutput")
        dVb = nc.dram_tensor("dVb", [P, 16, HEADS, D + 8], bf16, kind="ExternalOutput")
        doutT = nc.dram_tensor("doutT", [P, KT4, HALF], bf16, kind="ExternalOutput")

    Exp = mybir.ActivationFunctionType.Exp
    mult = mybir.AluOpType.mult
    add = mybir.AluOpType.add

    with tile.TileContext(nc) as tc, ExitStack() as ctx:
        big = ctx.enter_context(tc.tile_pool(name="big", bufs=4))
        ptp = ctx.enter_context(tc.tile_pool(name="ptp", bufs=8))
        wp = ctx.enter_context(tc.tile_pool(name="wp", bufs=1))
        pers = ctx.enter_context(tc.tile_pool(name="pers", bufs=1))
        yp = ctx.enter_context(tc.tile_pool(name="yp", bufs=2))
        nrm = ctx.enter_context(tc.tile_pool(name="nrm", bufs=2))
        # PSUM: tag "s" 3 x [128,1024] slots (6 banks) + tag "pv" 2 x 1 bank
        ps = ctx.enter_context(tc.tile_pool(name="ps", bufs=3, space="PSUM"))

        # ---- input loads, ordered to match first-use ----
        # round kt: wq[kt], wk[kt], xt[kt] feed the kt-outer startup matmuls
        xt = []
        wq_s = wp.tile([P, KT4, EMB], bf16, name="wqs", tag="wqs")
        wk_s = wp.tile([P, KT4, EMB], bf16, name="wks", tag="wks")
        wv_s = wp.tile([P, KT4, EMB], bf16, name="wvs", tag="wvs")
        wo_s = wp.tile([P, KT4, EMB], bf16, name="wos", tag="wos")
        for kt in range(KT4):
            nc.sync.dma_start(wq_s[:, kt], wq_d[kt])
            nc.sync.dma_start(wk_s[:, kt], wk_d[kt])
            t = big.tile([P, N], bf16, name=f"xt{kt}", tag="big")
            nc.sync.dma_start(t[:], xT_d[kt])
            xt.append(t)
        for kt in range(KT4):
            nc.sync.dma_start(wv_s[:, kt], wv_d[kt])
        for kt in range(KT4):
            nc.sync.dma_start(wo_s[:, kt], wo_d[kt])
        bq_s = pers.tile([P, KT4], dt, name="bqs")
        nc.sync.dma_start(bq_s[:], bq_d[:])
        bk_s = pers.tile([P, KT4], dt, name="bks")
        nc.sync.dma_start(bk_s[:], bk_d[:])
        bvr_s = pers.tile([P, HEADS, D], dt, name="bvrs")
        nc.sync.dma_start(bvr_s[:], bvr_d.ap().rearrange("p (h d) -> p h d", d=D))
        bor_s = pers.tile([P, EMB], dt, name="bors")
        nc.sync.dma_start(bor_s[:], bor_d[:])
        selA_s = pers.tile([1, P], bf16, name="selAs")
        nc.sync.dma_start(selA_s[:], sel_d[0:1])
        selB_s = pers.tile([1, P], bf16, name="selBs")
        nc.sync.dma_start(selB_s[:], sel_d[1:2])

        # ---- persistent intermediates ----
        QT = pers.tile([P, KT4, HALF], bf16, name="QT")
        KTt = pers.tile([P, KT4, N], bf16, name="KTt")
        Vb = pers.tile([P, 16, HEADS, D + 8], bf16, name="Vb")
        outT = pers.tile([P, KT4, HALF], bf16, name="outT")

        # single ones column per head: PV lands the softmax denominator on
        # PSUM partition 64 (aligned for the base-shifting drain copy)
        nc.vector.memset(Vb[:, :, :, D:D + 8], 0.0)
        nc.vector.memset(Vb[:, :, :, D], 1.0)

        # ---- startup: Q(0) and K(0) kt-outer so PE starts on first DMAs ----
        sQ = ps.tile([P, 1024], dt, tag="s", name="sQ0")
        sK1 = ps.tile([P, 1024], dt, tag="s", name="sK01")
        sK2 = ps.tile([P, 1024], dt, tag="s", name="sK23")
        for kt in range(KT4):
            for c in range(2):
                nc.tensor.matmul(
                    sQ[:, c * 512:(c + 1) * 512],
                    lhsT=wq_s[:, kt, 0:P],
                    rhs=xt[kt][:, c * 512:(c + 1) * 512],
                    start=kt == 0,
                    stop=kt == KT4 - 1,
                )
            for c in range(4):
                sK = sK1 if c < 2 else sK2
                nc.tensor.matmul(
                    sK[:, (c % 2) * 512:(c % 2 + 1) * 512],
                    lhsT=wk_s[:, kt, 0:P],
                    rhs=xt[kt][:, c * 512:(c + 1) * 512],
                    start=kt == 0,
                    stop=kt == KT4 - 1,
                )
        for c in range(2):
            nc.vector.tensor_scalar_add(
                QT[:, 0, c * 512:(c + 1) * 512],
                sQ[:, c * 512:(c + 1) * 512], bq_s[:, 0:1],
            )
        for c in range(4):
            sK = sK1 if c < 2 else sK2
            nc.vector.tensor_scalar_add(
                KTt[:, 0, c * 512:(c + 1) * 512],
                sK[:, (c % 2) * 512:(c % 2 + 1) * 512], bk_s[:, 0:1],
            )

        def emit_q(jt):
            for c in range(2):
                pq = ps.tile([P, 512], dt, tag="s", name=f"pq{jt}{c}")
                for kt in range(KT4):
                    nc.tensor.matmul(
                        pq[:],
                        lhsT=wq_s[:, kt, jt * P:(jt + 1) * P],
                        rhs=xt[kt][:, c * 512:(c + 1) * 512],
                        start=kt == 0,
                        stop=kt == KT4 - 1,
                    )
                nc.vector.tensor_scalar_add(
                    QT[:, jt, c * 512:(c + 1) * 512], pq[:], bq_s[:, jt:jt + 1]
                )

        def emit_k(jt):
            for c in range(4):
                pk = ps.tile([P, 512], dt, tag="s", name=f"pk{jt}{c}")
                for kt in range(KT4):
                    nc.tensor.matmul(
                        pk[:],
                        lhsT=wk_s[:, kt, jt * P:(jt + 1) * P],
                        rhs=xt[kt][:, c * 512:(c + 1) * 512],
                        start=kt == 0,
                        stop=kt == KT4 - 1,
                    )
                nc.vector.tensor_scalar_add(
                    KTt[:, jt, c * 512:(c + 1) * 512], pk[:], bk_s[:, jt:jt + 1]
                )

        def emit_v_tile(t):
            pv = ps.tile([P, 512], dt, tag="s", name=f"pvv{t}")
            for kt in range(KT4):
                nc.tensor.matmul(
                    pv[:],
                    lhsT=xt[kt][:, t * P:(t + 1) * P],
                    rhs=wv_s[:, kt, :],
                    start=kt == 0,
                    stop=kt == KT4 - 1,
                )
            nc.vector.tensor_tensor(
                Vb[:, t, :, 0:D],
                pv.rearrange("p (h d) -> p h d", d=D),
                bvr_s[:],
                add,
            )

        def emit_attn(hp, c, with_v=False):
            hA, hB = 2 * hp, 2 * hp + 1
            jt = hp  # feature tile holding this head pair
            cs = slice(c * 512, (c + 1) * 512)
            # one 2-bank PSUM tile: head A accumulates in cols 0:512, head B
            # in cols 512:1024; row 64 of each half is the softmax denominator
            pvM = ps.tile([D + 1, 1024], dt, tag="pv", bufs=1, name=f"pvM{hp}{c}")
            for t in range(16):
                # one PSUM tile holds the head pair's scores for key-tile t:
                # head A on PE rows 0:64 -> cols 0:512, head B on rows 64:128
                # -> cols 512:1024, emitted adjacently for row-tile concurrency
                sM = ps.tile([P, 1024], dt, tag="s", name=f"sM{hp}{c}{t}")
                nc.tensor.matmul(
                    sM[:, 0:512],
                    lhsT=KTt[0:D, jt, t * P:(t + 1) * P],
                    rhs=QT[0:D, jt, cs],
                    start=True,
                    stop=True,
                )
                nc.tensor.matmul(
                    sM[:, 512:1024],
                    lhsT=KTt[D:P, jt, t * P:(t + 1) * P],
                    rhs=QT[D:P, jt, cs],
                    start=True,
                    stop=True,
                )
                ptM = ptp.tile([P, 1024], bf16, tag="pt", name=f"pt{hp}{c}{t}")
                nc.scalar.activation(ptM[:], sM[:], Exp, scale=SCALE)
                if with_v:
                    emit_v_tile(t)
                nc.tensor.matmul(
                    pvM[:, 0:512],
                    lhsT=Vb[:, t, hA, 0:D + 1],
                    rhs=ptM[:, 0:512],
                    start=t == 0,
                    stop=t == 15,
                )
                nc.tensor.matmul(
                    pvM[:, 512:1024],
                    lhsT=Vb[:, t, hB, 0:D + 1],
                    rhs=ptM[:, 512:1024],
                    start=t == 0,
                    stop=t == 15,
                )
            # ---- fused drain + normalization for this head pair ----
            # denominators live on PSUM row 64 (cols 0:512 head A, 512:1024
            # head B); broadcast 1/denom to the 128 feature rows via two
            # 1-deep PE matmuls against constant 0/1 selectors
            r0 = nrm.tile([1, 1024], dt, tag="r0", name=f"r0{hp}{c}")
            nc.vector.tensor_copy(r0[:], pvM[D:D + 1, :])
            r1 = nrm.tile([1, 1024], dt, tag="r1", name=f"r1{hp}{c}")
            nc.vector.reciprocal_approx_fast(r1[:], r0[:])
            rb = nrm.tile([1, 1024], bf16, tag="rb", name=f"rb{hp}{c}")
            nc.vector.tensor_copy(rb[:], r1[:])
            pr = ps.tile([P, 512], dt, tag="s", name=f"pr{hp}{c}")
            nc.tensor.matmul(pr[:], lhsT=selA_s[:], rhs=rb[0:1, 0:512],
                             start=True, stop=False)
            nc.tensor.matmul(pr[:], lhsT=selB_s[:], rhs=rb[0:1, 512:1024],
                             start=False, stop=True)
            # copy each head's 64 output rows to SBUF, then scale by the
            # broadcast reciprocal (TT allows only one PSUM input, and needs
            # matching partition bases)
            nc.vector.tensor_copy(outT[0:D, hp, cs], pvM[0:D, 0:512])
            nc.vector.tensor_tensor(
                outT[0:D, hp, cs], outT[0:D, hp, cs], pr[0:D, :], mult
            )
            nc.vector.tensor_copy(outT[D:P, hp, cs], pvM[0:D, 512:1024])
            nc.vector.tensor_tensor(
                outT[D:P, hp, cs], outT[D:P, hp, cs], pr[D:P, :], mult
            )

        def emit_oproj(c):
            for mm in range(4):
                m = 4 * c + mm
                py = ps.tile([P, 512], dt, tag="s", name=f"py{m}")
                for et in range(KT4):
                    nc.tensor.matmul(
                        py[:],
                        lhsT=outT[:, et, m * P:(m + 1) * P],
                        rhs=wo_s[:, et, :],
                        start=et == 0,
                        stop=et == KT4 - 1,
                    )
                yt = yp.tile([P, 512], dt, tag="y", name=f"yt{m}")
                nc.vector.tensor_tensor(yt[:], py[:], bor_s[:], add)
                nc.sync.dma_start(y_d[m * P:(m + 1) * P, :], yt[:])

        # chunk-outer: chunk 0's output projection + DMA overlap chunk 1's
        # ScalarE-bound attention.  Q/K projections for head-pair hp+1 are
        # interleaved after attention on hp (PE has slack while ScalarE works
        # through the exp stream).  oproj(0) is emitted after attn(0,1) so the
        # c=1 score matmuls take scheduling priority at the chunk boundary.
        for hp in range(4):
            emit_attn(hp, 0, with_v=(hp == 0))
            if hp + 1 < KT4:
                emit_q(hp + 1)
                emit_k(hp + 1)
        for hp in range(4):
            emit_attn(hp, 1)
            if hp == 0:
                emit_oproj(0)
        emit_oproj(1)

        if debug:
            nc.sync.dma_start(dQT.ap(), QT[:])
            nc.sync.dma_start(dKT.ap(), KTt[:])
            nc.sync.dma_start(dVb.ap(), Vb[:])
            nc.sync.dma_start(doutT.ap(), outT[:])

    nc.finalize()
    return nc


def _get_program(debug=False):
    key = ("nc", debug)
    if key not in _CACHE:
        _CACHE[key] = _build_program(debug)
    return _CACHE[key]


def _host_inputs(x, Wq, bq, Wk, bk, Wv, bv, Wo, bo):
    import ml_dtypes
    f32 = np.float32
    bf = ml_dtypes.bfloat16
    wqT = np.ascontiguousarray(np.asarray(Wq, f32).T).reshape(KT4, P, EMB).astype(bf)
    wkT = np.ascontiguousarray(np.asarray(Wk, f32).T).reshape(KT4, P, EMB).astype(bf)
    wvT = np.ascontiguousarray(np.asarray(Wv, f32).T).reshape(KT4, P, EMB).astype(bf)
    woT = np.ascontiguousarray(np.asarray(Wo, f32).T).reshape(KT4, P, EMB).astype(bf)
    bq2 = np.ascontiguousarray(np.asarray(bq, f32).reshape(KT4, P).T)
    bk2 = np.ascontiguousarray(np.asarray(bk, f32).reshape(KT4, P).T)
    bvr = np.ascontiguousarray(np.tile(np.asarray(bv, f32), (P, 1)))
    bor = np.ascontiguousarray(np.tile(np.asarray(bo, f32), (P, 1)))
    sel2 = np.zeros((2, P), f32)
    for m in range(P):
        sel2[m // D, m] = 1.0
    sel2 = sel2.astype(bf)

    shared = dict(wq=wqT, wk=wkT, wv=wvT, wo=woT, bq2=bq2, bk2=bk2,
                  bvr=bvr, bor=bor, sel2=sel2)
    x = np.asarray(x, f32)
    in_maps = []
    for c in range(NCORES):
        b, hf = c // 2, c % 2
        xb = x[b]
        # queries first; key order is irrelevant as long as K and V agree
        xr = np.concatenate(
            [xb[hf * HALF:(hf + 1) * HALF], xb[(1 - hf) * HALF:(2 - hf) * HALF]], 0
        )
        xT = np.ascontiguousarray(xr.T).reshape(KT4, P, N).astype(bf)
        in_maps.append(dict(shared, xT=xT))
    return in_maps


def kernel(x, Wq, bq, Wk, bk, Wv, bv, Wo, bo, _trace=False, _trace_cores=None,
           _debug=False):
    from concourse.bass_utils import run_bass_kernel_spmd

    nc = _get_program(_debug)
    in_maps = _host_inputs(x, Wq, bq, Wk, bk, Wv, bv, Wo, bo)
    res = run_bass_kernel_spmd(
        nc, in_maps, list(range(NCORES)), trace=_trace,
        trace_cores=_trace_cores,
    )
    out = np.empty((B, N, EMB), np.float32)
    for c in range(NCORES):
        b, hf = c // 2, c % 2
        out[b, hf * HALF:(hf + 1) * HALF] = res.results[c]["y"]
    if _trace:
        _CACHE["last_results"] = res
    return out


# revision 18
# speedup vs baseline: 1.0632x; 1.0632x over previous
"""Multi-head attention (EMB=512, HEADS=8, x:(4,2048,512)) on 8 Trainium2 cores.

Sharding: zero-collective split — core c handles batch c//2, query rows
(c%2)*1024..(c%2+1)*1024, ALL heads.  K/V projections for the full batch are
computed redundantly on the 2 cores sharing a batch (no collectives at all).

All matmul operands are bf16: on TRN2 a 128-deep-contraction fp32r matmul
runs 2 cycles/row ("HIGH" replicated mode) while bf16 runs 1 cycle/row with
FWL weight loads, so bf16 halves projection time and halves input DMA bytes.

Device-side dataflow (per core, everything SBUF-resident):
  xT (host-transposed)           [512, 2048]   keys reordered so queries first
  Q^T = WqT.T @ xT  (+bq)        [512, 1024]   feature-major (bf16)
  K^T = WkT.T @ xT  (+bk)        [512, 2048]   feature-major (bf16)
  V~  = xT.T @ WvT  (+bv, ones)  [2048, 8*72]  token-major, per-head ones col
  S^T(t) = K^T_h.T @ Q^T_h       one [128,1024] PSUM tile per key-tile t holds
                                 BOTH heads of the pair: cols 0:512 head A on
                                 PE rows 0:64, cols 512:1024 head B on rows
                                 64:128 — emitted adjacently so the pair runs
                                 concurrently via PE row tiling
  P^T = exp(S^T / sqrt(512))     (ScalarE, fused drain from PSUM -> bf16)
  outT~ = V~_h.T @ P^T           [72, 512]     row 64+h = softmax denominator
  outT = outT~ * bcast(1/denom)  normalization fused into the PSUM drain
  y = outT.T @ WoT (+bo)         [1024, 512]   token-major, DMA out

Loop order: query-chunk c is OUTER so chunk 0's output projection + DMA
overlap chunk 1's (ScalarE-bound) attention.  Startup is kt-outer so the
first Q/K projections overlap the input DMA stream.  V-projection tiles are
emitted inside the first attention block's t-loop, just in time for PV.
"""

import sys
import os

for _p in ("/opt/trn_rl_repo", "/root/.axon_site/_ro/trn_rl_repo"):
    if os.path.isdir(_p) and _p not in sys.path:
        sys.path.append(_p)

import numpy as np

EMB = 512
HEADS = 8
D = 64  # head dim
B = 4
N = 2048  # keys / tokens per batch
HALF = 1024  # queries per core
P = 128
NCORES = 8
KT4 = EMB // P  # 4 contraction tiles
SCALE = float(1.0 / np.sqrt(np.float32(EMB)))

_CACHE = {}


def _build_program(debug=False):
    from concourse import bacc
    import concourse.mybir as mybir
    import concourse.tile as tile
    from contextlib import ExitStack

    dt = mybir.dt.float32
    bf16 = mybir.dt.bfloat16
    nc = bacc.Bacc("TRN2", target_bir_lowering=False)

    xT_d = nc.dram_tensor("xT", [KT4, P, N], bf16, kind="ExternalInput")
    wq_d = nc.dram_tensor("wq", [KT4, P, EMB], bf16, kind="ExternalInput")
    wk_d = nc.dram_tensor("wk", [KT4, P, EMB], bf16, kind="ExternalInput")
    wv_d = nc.dram_tensor("wv", [KT4, P, EMB], bf16, kind="ExternalInput")
    wo_d = nc.dram_tensor("wo", [KT4, P, EMB], bf16, kind="ExternalInput")
    bq_d = nc.dram_tensor("bq2", [P, KT4], dt, kind="ExternalInput")
    bk_d = nc.dram_tensor("bk2", [P, KT4], dt, kind="ExternalInput")
    bvr_d = nc.dram_tensor("bvr", [P, EMB], dt, kind="ExternalInput")
    bor_d = nc.dram_tensor("bor", [P, EMB], dt, kind="ExternalInput")
    sel_d = nc.dram_tensor("sel2", [2, P], bf16, kind="ExternalInput")  # row 0: p<64, row 1: p>=64
    y_d = nc.dram_tensor("y", [HALF, EMB], dt, kind="ExternalOutput")
    if debug:
        dQT = nc.dram_tensor("dQT", [P, KT4, HALF], bf16, kind="ExternalOutput")
        dKT = nc.dram_tensor("dKT", [P, KT4, N], bf16, kind="ExternalOutput")
        dVb = nc.dram_tensor("dVb", [P, 16, HEADS, D + 8], bf16, kind="ExternalOutput")
        doutT = nc.dram_tensor("doutT", [P, KT4, HALF], bf16, kind="ExternalOutput")

    Exp = mybir.ActivationFunctionType.Exp
    mult = mybir.AluOpType.mult
    add = mybir.AluOpType.add

    with tile.TileContext(nc) as tc, ExitStack() as ctx:
        big = ctx.enter_context(tc.tile_pool(name="big", bufs=4))
        ptp = ctx.enter_context(tc.tile_pool(name="ptp", bufs=8))
        wp = ctx.enter_context(tc.tile_pool(name="wp", bufs=1))
        pers = ctx.enter_context(tc.tile_pool(name="pers", bufs=1))
        yp = ctx.enter_context(tc.tile_pool(name="yp", bufs=2))
        nrm = ctx.enter_context(tc.tile_pool(name="nrm", bufs=2))
        # PSUM: tag "s" 3 x [128,1024] slots (6 banks) + tag "pv" 2 x 1 bank
        ps = ctx.enter_context(tc.tile_pool(name="ps", bufs=3, space="PSUM"))

        # ---- input loads, ordered to match first-use ----
        # round kt: wq[kt], wk[kt], xt[kt] feed the kt-outer startup matmuls
        xt = []
        wq_s = wp.tile([P, KT4, EMB], bf16, name="wqs", tag="wqs")
        wk_s = wp.tile([P, KT4, EMB], bf16, name="wks", tag="wks")
        wv_s = wp.tile([P, KT4, EMB], bf16, name="wvs", tag="wvs")
        wo_s = wp.tile([P, KT4, EMB], bf16, name="wos", tag="wos")
        for kt in range(KT4):
            nc.sync.dma_start(wq_s[:, kt], wq_d[kt])
            nc.sync.dma_start(wk_s[:, kt], wk_d[kt])
            t = big.tile([P, N], bf16, name=f"xt{kt}", tag="big")
            nc.sync.dma_start(t[:], xT_d[kt])
            xt.append(t)
        for kt in range(KT4):
            nc.sync.dma_start(wv_s[:, kt], wv_d[kt])
        for kt in range(KT4):
            nc.sync.dma_start(wo_s[:, kt], wo_d[kt])
        bq_s = pers.tile([P, KT4], dt, name="bqs")
        nc.sync.dma_start(bq_s[:], bq_d[:])
        bk_s = pers.tile([P, KT4], dt, name="bks")
        nc.sync.dma_start(bk_s[:], bk_d[:])
        bvr_s = pers.tile([P, HEADS, D], dt, name="bvrs")
        nc.sync.dma_start(bvr_s[:], bvr_d.ap().rearrange("p (h d) -> p h d", d=D))
        bor_s = pers.tile([P, EMB], dt, name="bors")
        nc.sync.dma_start(bor_s[:], bor_d[:])
        selA_s = pers.tile([1, P], bf16, name="selAs")
        nc.sync.dma_start(selA_s[:], sel_d[0:1])
        selB_s = pers.tile([1, P], bf16, name="selBs")
        nc.sync.dma_start(selB_s[:], sel_d[1:2])

        # ---- persistent intermediates ----
        QT = pers.tile([P, KT4, HALF], bf16, name="QT")
        KTt = pers.tile([P, KT4, N], bf16, name="KTt")
        Vb = pers.tile([P, 16, HEADS, D + 8], bf16, name="Vb")
        outT = pers.tile([P, KT4, HALF], bf16, name="outT")

        # single ones column per head: PV lands the softmax denominator on
        # PSUM partition 64 (aligned for the base-shifting drain copy)
        nc.vector.memset(Vb[:, :, :, D:D + 8], 0.0)
        nc.vector.memset(Vb[:, :, :, D], 1.0)

        # ---- startup: Q(0) and K(0) kt-outer so PE starts on first DMAs ----
        sQ = ps.tile([P, 1024], dt, tag="s", name="sQ0")
        sK1 = ps.tile([P, 1024], dt, tag="s", name="sK01")
        sK2 = ps.tile([P, 1024], dt, tag="s", name="sK23")
        for kt in range(KT4):
            for c in range(2):
                nc.tensor.matmul(
                    sQ[:, c * 512:(c + 1) * 512],
                    lhsT=wq_s[:, kt, 0:P],
                    rhs=xt[kt][:, c * 512:(c + 1) * 512],
                    start=kt == 0,
                    stop=kt == KT4 - 1,
                )
            for c in range(4):
                sK = sK1 if c < 2 else sK2
                nc.tensor.matmul(
                    sK[:, (c % 2) * 512:(c % 2 + 1) * 512],
                    lhsT=wk_s[:, kt, 0:P],
                    rhs=xt[kt][:, c * 512:(c + 1) * 512],
                    start=kt == 0,
                    stop=kt == KT4 - 1,
                )
        for c in range(2):
            nc.vector.tensor_scalar_add(
                QT[:, 0, c * 512:(c + 1) * 512],
                sQ[:, c * 512:(c + 1) * 512], bq_s[:, 0:1],
            )
        for c in range(4):
            sK = sK1 if c < 2 else sK2
            nc.vector.tensor_scalar_add(
                KTt[:, 0, c * 512:(c + 1) * 512],
                sK[:, (c % 2) * 512:(c % 2 + 1) * 512], bk_s[:, 0:1],
            )

        def q_tile(jt, c):
            def emit():
                pq = ps.tile([P, 512], dt, tag="s", name=f"pq{jt}{c}")
                for kt in range(KT4):
                    nc.tensor.matmul(
                        pq[:],
                        lhsT=wq_s[:, kt, jt * P:(jt + 1) * P],
                        rhs=xt[kt][:, c * 512:(c + 1) * 512],
                        start=kt == 0,
                        stop=kt == KT4 - 1,
                    )
                nc.vector.tensor_scalar_add(
                    QT[:, jt, c * 512:(c + 1) * 512], pq[:], bq_s[:, jt:jt + 1]
                )
            return emit

        def k_tile(jt, c):
            def emit():
                pk = ps.tile([P, 512], dt, tag="s", name=f"pk{jt}{c}")
                for kt in range(KT4):
                    nc.tensor.matmul(
                        pk[:],
                        lhsT=wk_s[:, kt, jt * P:(jt + 1) * P],
                        rhs=xt[kt][:, c * 512:(c + 1) * 512],
                        start=kt == 0,
                        stop=kt == KT4 - 1,
                    )
                nc.vector.tensor_scalar_add(
                    KTt[:, jt, c * 512:(c + 1) * 512], pk[:], bk_s[:, jt:jt + 1]
                )
            return emit

        def qk_fill(jt):
            return [q_tile(jt, 0), q_tile(jt, 1)] + [k_tile(jt, c) for c in range(4)]

        def emit_v_tile(t):
            pv = ps.tile([P, 512], dt, tag="s", name=f"pvv{t}")
            for kt in range(KT4):
                nc.tensor.matmul(
                    pv[:],
                    lhsT=xt[kt][:, t * P:(t + 1) * P],
                    rhs=wv_s[:, kt, :],
                    start=kt == 0,
                    stop=kt == KT4 - 1,
                )
            nc.vector.tensor_tensor(
                Vb[:, t, :, 0:D],
                pv.rearrange("p (h d) -> p h d", d=D),
                bvr_s[:],
                add,
            )

        def emit_attn(hp, c, with_v=False, fill=(), drain_prev=None):
            """fill: closures emitting extra PE work, consumed one per t-slot
            starting at the back; drain_prev: previous block's deferred drain,
            emitted at t==2 so it never head-of-line-blocks this block's score
            matmuls."""
            hA, hB = 2 * hp, 2 * hp + 1
            jt = hp  # feature tile holding this head pair
            cs = slice(c * 512, (c + 1) * 512)
            # one 2-bank PSUM tile: head A accumulates in cols 0:512, head B
            # in cols 512:1024; row 64 of each half is the softmax denominator
            pvM = ps.tile([D + 1, 1024], dt, tag="pv", bufs=1, name=f"pvM{hp}{c}")
            fill = list(fill)
            # spread fill work over t-slots: late slots when V occupies the
            # early ones, else evenly from t=3
            if with_v:
                fill_slots = set(range(16 - len(fill), 16))
            else:
                step = max(1, 13 // max(1, len(fill)))
                fill_slots = set(range(3, 16, step))
            for t in range(16):
                # one PSUM tile holds the head pair's scores for key-tile t:
                # head A on PE rows 0:64 -> cols 0:512, head B on rows 64:128
                # -> cols 512:1024, emitted adjacently for row-tile concurrency
                sM = ps.tile([P, 1024], dt, tag="s", name=f"sM{hp}{c}{t}")
                nc.tensor.matmul(
                    sM[:, 0:512],
                    lhsT=KTt[0:D, jt, t * P:(t + 1) * P],
                    rhs=QT[0:D, jt, cs],
                    start=True,
                    stop=True,
                )
                nc.tensor.matmul(
                    sM[:, 512:1024],
                    lhsT=KTt[D:P, jt, t * P:(t + 1) * P],
                    rhs=QT[D:P, jt, cs],
                    start=True,
                    stop=True,
                )
                ptM = ptp.tile([P, 1024], bf16, tag="pt", name=f"pt{hp}{c}{t}")
                nc.scalar.activation(ptM[:], sM[:], Exp, scale=SCALE)
                if with_v:
                    emit_v_tile(t)
                if t == 2 and drain_prev is not None:
                    drain_prev()
                if fill and t in fill_slots:
                    fill.pop(0)()
                nc.tensor.matmul(
                    pvM[:, 0:512],
                    lhsT=Vb[:, t, hA, 0:D + 1],
                    rhs=ptM[:, 0:512],
                    start=t == 0,
                    stop=t == 15,
                )
                nc.tensor.matmul(
                    pvM[:, 512:1024],
                    lhsT=Vb[:, t, hB, 0:D + 1],
                    rhs=ptM[:, 512:1024],
                    start=t == 0,
                    stop=t == 15,
                )
            assert not fill

            def drain():
                # denominators live on PSUM row 64 (cols 0:512 head A,
                # 512:1024 head B); broadcast 1/denom to the 128 feature rows
                # via two 1-deep PE matmuls against constant 0/1 selectors
                r0 = nrm.tile([1, 1024], dt, tag="r0", name=f"r0{hp}{c}")
                nc.vector.tensor_copy(r0[:], pvM[D:D + 1, :])
                r1 = nrm.tile([1, 1024], dt, tag="r1", name=f"r1{hp}{c}")
                nc.vector.reciprocal_approx_fast(r1[:], r0[:])
                rb = nrm.tile([1, 1024], bf16, tag="rb", name=f"rb{hp}{c}")
                nc.vector.tensor_copy(rb[:], r1[:])
                pr = ps.tile([P, 512], dt, tag="s", name=f"pr{hp}{c}")
                nc.tensor.matmul(pr[:], lhsT=selA_s[:], rhs=rb[0:1, 0:512],
                                 start=True, stop=False)
                nc.tensor.matmul(pr[:], lhsT=selB_s[:], rhs=rb[0:1, 512:1024],
                                 start=False, stop=True)
                # copy each head's 64 output rows to SBUF, then scale by the
                # broadcast reciprocal (TT allows only one PSUM input, and
                # needs matching partition bases)
                nc.vector.tensor_copy(outT[0:D, hp, cs], pvM[0:D, 0:512])
                nc.vector.tensor_tensor(
                    outT[0:D, hp, cs], outT[0:D, hp, cs], pr[0:D, :], mult
                )
                nc.vector.tensor_copy(outT[D:P, hp, cs], pvM[0:D, 512:1024])
                nc.vector.tensor_tensor(
                    outT[D:P, hp, cs], outT[D:P, hp, cs], pr[D:P, :], mult
                )
            return drain

        def o_tile(m):
            def emit():
                py = ps.tile([P, 512], dt, tag="s", name=f"py{m}")
                for et in range(KT4):
                    nc.tensor.matmul(
                        py[:],
                        lhsT=outT[:, et, m * P:(m + 1) * P],
                        rhs=wo_s[:, et, :],
                        start=et == 0,
                        stop=et == KT4 - 1,
                    )
                yt = yp.tile([P, 512], dt, tag="y", name=f"yt{m}")
                nc.vector.tensor_tensor(yt[:], py[:], bor_s[:], add)
                nc.sync.dma_start(y_d[m * P:(m + 1) * P, :], yt[:])
            return emit

        # chunk-outer: chunk 0's output projection + DMA overlap chunk 1's
        # ScalarE-bound attention.  Q/K projections for head-pair hp+1 and
        # chunk 0's output projection are spread over t-slots of later blocks
        # (the PE has slack while ScalarE works through the exp stream), and
        # each block's drain is deferred into the next block so its recip
        # chain never head-of-line-blocks the score matmuls.
        fills = {
            (0, 0): qk_fill(1),
            (1, 0): qk_fill(2),
            (2, 0): qk_fill(3),
            (1, 1): [o_tile(m) for m in range(4)],
        }
        drain = None
        for c in range(2):
            for hp in range(4):
                drain = emit_attn(hp, c, with_v=(c == 0 and hp == 0),
                                  fill=fills.get((hp, c), ()),
                                  drain_prev=drain)
        drain()
        for m in range(4, 8):
            o_tile(m)()

        if debug:
            nc.sync.dma_start(dQT.ap(), QT[:])
            nc.sync.dma_start(dKT.ap(), KTt[:])
            nc.sync.dma_start(dVb.ap(), Vb[:])
            nc.sync.dma_start(doutT.ap(), outT[:])

    nc.finalize()
    return nc


def _get_program(debug=False):
    key = ("nc", debug)
    if key not in _CACHE:
        _CACHE[key] = _build_program(debug)
    return _CACHE[key]


def _host_inputs(x, Wq, bq, Wk, bk, Wv, bv, Wo, bo):
    import ml_dtypes
    f32 = np.float32
    bf = ml_dtypes.bfloat16
    wqT = np.ascontiguousarray(np.asarray(Wq, f32).T).reshape(KT4, P, EMB).astype(bf)
    wkT = np.ascontiguousarray(np.asarray(Wk, f32).T).reshape(KT4, P, EMB).astype(bf)
    wvT = np.ascontiguousarray(np.asarray(Wv, f32).T).reshape(KT4, P, EMB).astype(bf)
    woT = np.ascontiguousarray(np.asarray(Wo, f32).T).reshape(KT4, P, EMB).astype(bf)
    bq2 = np.ascontiguousarray(np.asarray(bq, f32).reshape(KT4, P).T)
    bk2 = np.ascontiguousarray(np.asarray(bk, f32).reshape(KT4, P).T)
    bvr = np.ascontiguousarray(np.tile(np.asarray(bv, f32), (P, 1)))
    bor = np.ascontiguousarray(np.tile(np.asarray(bo, f32), (P, 1)))
    sel2 = np.zeros((2, P), f32)
    for m in range(P):
        sel2[m // D, m] = 1.0
    sel2 = sel2.astype(bf)

    shared = dict(wq=wqT, wk=wkT, wv=wvT, wo=woT, bq2=bq2, bk2=bk2,
                  bvr=bvr, bor=bor, sel2=sel2)
    x = np.asarray(x, f32)
    in_maps = []
    for c in range(NCORES):
        b, hf = c // 2, c % 2
        xb = x[b]
        # queries first; key order is irrelevant as long as K and V agree
        xr = np.concatenate(
            [xb[hf * HALF:(hf + 1) * HALF], xb[(1 - hf) * HALF:(2 - hf) * HALF]], 0
        )
        xT = np.ascontiguousarray(xr.T).reshape(KT4, P, N).astype(bf)
        in_maps.append(dict(shared, xT=xT))
    return in_maps


def kernel(x, Wq, bq, Wk, bk, Wv, bv, Wo, bo, _trace=False, _trace_cores=None,
           _debug=False):
    from concourse.bass_utils import run_bass_kernel_spmd

    nc = _get_program(_debug)
    in_maps = _host_inputs(x, Wq, bq, Wk, bk, Wv, bv, Wo, bo)
    res = run_bass_kernel_spmd(
        nc, in_maps, list(range(NCORES)), trace=_trace,
        trace_cores=_trace_cores,
    )
    out = np.empty((B, N, EMB), np.float32)
    for c in range(NCORES):
        b, hf = c // 2, c % 2
        out[b, hf * HALF:(hf + 1) * HALF] = res.results[c]["y"]
    if _trace:
        _CACHE["last_results"] = res
    return out


# revision 26
# speedup vs baseline: 1.2308x; 1.1576x over previous
"""Multi-head attention (EMB=512, HEADS=8, x:(4,2048,512)) on 8 Trainium2 cores.

Sharding: zero-collective split — core c handles batch c//2, query rows
(c%2)*1024..(c%2+1)*1024, ALL heads.  K/V projections for the full batch are
computed redundantly on the 2 cores sharing a batch (no collectives at all).

All matmul operands are bf16: on TRN2 a 128-deep-contraction fp32r matmul
runs 2 cycles/row ("HIGH" replicated mode) while bf16 runs 1 cycle/row with
FWL weight loads, so bf16 halves projection time and halves input DMA bytes.

Device-side dataflow (per core, everything SBUF-resident):
  xT (host-transposed)           [512, 2048]   keys reordered so queries first
  Q^T = WqT.T @ xT  (+bq)        [512, 1024]   feature-major (bf16)
  K^T = WkT.T @ xT  (+bk)        [512, 2048]   feature-major (bf16)
  V~  = xT.T @ WvT  (+bv, ones)  [2048, 8*72]  token-major, per-head ones col
  S^T(t) = K^T_h.T @ Q^T_h       one [128,1024] PSUM tile per key-tile t holds
                                 BOTH heads of the pair: cols 0:512 head A on
                                 PE rows 0:64, cols 512:1024 head B on rows
                                 64:128 — emitted adjacently so the pair runs
                                 concurrently via PE row tiling
  P^T = exp(S^T / sqrt(512))     (ScalarE, fused drain from PSUM -> bf16)
  outT~ = V~_h.T @ P^T           [72, 512]     row 64+h = softmax denominator
  outT = outT~ * bcast(1/denom)  normalization fused into the PSUM drain
  y = outT.T @ WoT (+bo)         [1024, 512]   token-major, DMA out

Loop order: query-chunk c is OUTER so chunk 0's output projection + DMA
overlap chunk 1's (ScalarE-bound) attention.  Startup is kt-outer so the
first Q/K projections overlap the input DMA stream.  V-projection tiles are
emitted inside the first attention block's t-loop, just in time for PV.
"""

import sys
import os

for _p in ("/opt/trn_rl_repo", "/root/.axon_site/_ro/trn_rl_repo"):
    if os.path.isdir(_p) and _p not in sys.path:
        sys.path.append(_p)

import numpy as np

EMB = 512
HEADS = 8
D = 64  # head dim
B = 4
N = 2048  # keys / tokens per batch
HALF = 1024  # queries per core
P = 128
NCORES = 8
KT4 = EMB // P  # 4 contraction tiles
SCALE = float(1.0 / np.sqrt(np.float32(EMB)))

_CACHE = {}


def _build_program(debug=False):
    from concourse import bacc
    import concourse.mybir as mybir
    import concourse.tile as tile
    from contextlib import ExitStack

    dt = mybir.dt.float32
    bf16 = mybir.dt.bfloat16
    nc = bacc.Bacc("TRN2", target_bir_lowering=False)

    xT_d = nc.dram_tensor("xT", [KT4, P, N], bf16, kind="ExternalInput")
    wq_d = nc.dram_tensor("wq", [KT4, P, EMB], bf16, kind="ExternalInput")
    wk_d = nc.dram_tensor("wk", [KT4, P, EMB], bf16, kind="ExternalInput")
    wv_d = nc.dram_tensor("wv", [KT4, P, EMB], bf16, kind="ExternalInput")
    wo_d = nc.dram_tensor("wo", [KT4, P, EMB], bf16, kind="ExternalInput")
    bq_d = nc.dram_tensor("bq2", [P, KT4], dt, kind="ExternalInput")
    bk_d = nc.dram_tensor("bk2", [P, KT4], dt, kind="ExternalInput")
    bvr_d = nc.dram_tensor("bvr", [P, EMB], dt, kind="ExternalInput")
    bor_d = nc.dram_tensor("bor", [P, EMB], dt, kind="ExternalInput")
    sel_d = nc.dram_tensor("sel2", [2, P], bf16, kind="ExternalInput")  # row 0: p<64, row 1: p>=64
    y_d = nc.dram_tensor("y", [HALF, EMB], dt, kind="ExternalOutput")
    if debug:
        dQT = nc.dram_tensor("dQT", [P, KT4, HALF], bf16, kind="ExternalOutput")
        dKT = nc.dram_tensor("dKT", [P, KT4, N], bf16, kind="ExternalOutput")
        dVb = nc.dram_tensor("dVb", [P, 16, HEADS, D + 8], bf16, kind="ExternalOutput")
        doutT = nc.dram_tensor("doutT", [P, KT4, HALF], bf16, kind="ExternalOutput")

    Exp = mybir.ActivationFunctionType.Exp
    mult = mybir.AluOpType.mult
    add = mybir.AluOpType.add

    with tile.TileContext(nc) as tc, ExitStack() as ctx:
        big = ctx.enter_context(tc.tile_pool(name="big", bufs=4))
        ptp = ctx.enter_context(tc.tile_pool(name="ptp", bufs=8))
        wp = ctx.enter_context(tc.tile_pool(name="wp", bufs=1))
        pers = ctx.enter_context(tc.tile_pool(name="pers", bufs=1))
        yp = ctx.enter_context(tc.tile_pool(name="yp", bufs=2))
        nrm = ctx.enter_context(tc.tile_pool(name="nrm", bufs=2))
        # PSUM: tag "s" 2 x [128,1024] score slots (4 banks) + tag "pv" 1 x
        # [65,1024] (2 banks) + tag "w" 2 x [128,512] (2 banks) for the
        # projection/broadcast tiles, so they never disturb the score-tile
        # rotation that paces ScalarE
        ps = ctx.enter_context(tc.tile_pool(name="ps", bufs=2, space="PSUM"))

        # ---- input loads, ordered to match first-use ----
        # tiny bias/selector tensors first (the startup drains need them),
        # then round kt: wq[kt], wk[kt], xt[kt] feeding the kt-outer startup
        bq_s = pers.tile([P, KT4], dt, name="bqs")
        nc.sync.dma_start(bq_s[:], bq_d[:])
        bk_s = pers.tile([P, KT4], dt, name="bks")
        nc.sync.dma_start(bk_s[:], bk_d[:])
        bvr_s = pers.tile([P, HEADS, D], dt, name="bvrs")
        nc.sync.dma_start(bvr_s[:], bvr_d.ap().rearrange("p (h d) -> p h d", d=D))
        bor_s = pers.tile([P, EMB], dt, name="bors")
        nc.sync.dma_start(bor_s[:], bor_d[:])
        selA_s = pers.tile([1, P], bf16, name="selAs")
        nc.sync.dma_start(selA_s[:], sel_d[0:1])
        selB_s = pers.tile([1, P], bf16, name="selBs")
        nc.sync.dma_start(selB_s[:], sel_d[1:2])
        xt = []
        wq_s = wp.tile([P, KT4, EMB], bf16, name="wqs", tag="wqs")
        wk_s = wp.tile([P, KT4, EMB], bf16, name="wks", tag="wks")
        wv_s = wp.tile([P, KT4, EMB], bf16, name="wvs", tag="wvs")
        wo_s = wp.tile([P, KT4, EMB], bf16, name="wos", tag="wos")
        for kt in range(KT4):
            nc.sync.dma_start(wq_s[:, kt], wq_d[kt])
            nc.sync.dma_start(wk_s[:, kt], wk_d[kt])
            t = big.tile([P, N], bf16, name=f"xt{kt}", tag="big")
            nc.sync.dma_start(t[:], xT_d[kt])
            xt.append(t)
        for kt in range(KT4):
            nc.sync.dma_start(wv_s[:, kt], wv_d[kt])
        for kt in range(KT4):
            nc.sync.dma_start(wo_s[:, kt], wo_d[kt])

        # ---- persistent intermediates ----
        QT = pers.tile([P, KT4, HALF], bf16, name="QT")
        KTt = pers.tile([P, KT4, N], bf16, name="KTt")
        Vb = pers.tile([P, 16, HEADS, D + 8], bf16, name="Vb")
        outT = pers.tile([P, KT4, HALF], bf16, name="outT")

        # single ones column per head: PV lands the softmax denominator on
        # PSUM partition 64 (aligned for the base-shifting drain copy)
        nc.vector.memset(Vb[:, :, :, D:D + 8], 0.0)
        nc.vector.memset(Vb[:, :, :, D], 1.0)

        # ---- startup: Q(0) and K(0) kt-outer so PE starts on first DMAs ----
        sQ = ps.tile([P, 1024], dt, tag="s", name="sQ0")
        sK1 = ps.tile([P, 1024], dt, tag="s", name="sK01")
        sK2 = ps.tile([P, 512], dt, tag="w", name="sK2")
        sK3 = ps.tile([P, 512], dt, tag="w", name="sK3")
        for kt in range(KT4):
            for c in range(2):
                nc.tensor.matmul(
                    sQ[:, c * 512:(c + 1) * 512],
                    lhsT=wq_s[:, kt, 0:P],
                    rhs=xt[kt][:, c * 512:(c + 1) * 512],
                    start=kt == 0,
                    stop=kt == KT4 - 1,
                )
            for c in range(4):
                sK = (sK1[:, 0:512], sK1[:, 512:1024], sK2[:], sK3[:])[c]
                nc.tensor.matmul(
                    sK,
                    lhsT=wk_s[:, kt, 0:P],
                    rhs=xt[kt][:, c * 512:(c + 1) * 512],
                    start=kt == 0,
                    stop=kt == KT4 - 1,
                )
        for c in range(2):
            nc.vector.tensor_scalar_add(
                QT[:, 0, c * 512:(c + 1) * 512],
                sQ[:, c * 512:(c + 1) * 512], bq_s[:, 0:1],
            )
        for c in range(4):
            sK = (sK1[:, 0:512], sK1[:, 512:1024], sK2[:], sK3[:])[c]
            nc.vector.tensor_scalar_add(
                KTt[:, 0, c * 512:(c + 1) * 512], sK, bk_s[:, 0:1],
            )

        def q_tile(jt, c):
            def emit():
                pq = ps.tile([P, 512], dt, tag="w", name=f"pq{jt}{c}")
                for kt in range(KT4):
                    nc.tensor.matmul(
                        pq[:],
                        lhsT=wq_s[:, kt, jt * P:(jt + 1) * P],
                        rhs=xt[kt][:, c * 512:(c + 1) * 512],
                        start=kt == 0,
                        stop=kt == KT4 - 1,
                    )
                nc.vector.tensor_scalar_add(
                    QT[:, jt, c * 512:(c + 1) * 512], pq[:], bq_s[:, jt:jt + 1]
                )
            return emit

        def k_tile(jt, c):
            def emit():
                pk = ps.tile([P, 512], dt, tag="w", name=f"pk{jt}{c}")
                for kt in range(KT4):
                    nc.tensor.matmul(
                        pk[:],
                        lhsT=wk_s[:, kt, jt * P:(jt + 1) * P],
                        rhs=xt[kt][:, c * 512:(c + 1) * 512],
                        start=kt == 0,
                        stop=kt == KT4 - 1,
                    )
                nc.vector.tensor_scalar_add(
                    KTt[:, jt, c * 512:(c + 1) * 512], pk[:], bk_s[:, jt:jt + 1]
                )
            return emit

        def qk_fill(jt):
            return [q_tile(jt, 0), q_tile(jt, 1)] + [k_tile(jt, c) for c in range(4)]

        def emit_v_tile(t):
            pv = ps.tile([P, 512], dt, tag="w", name=f"pvv{t}")
            for kt in range(KT4):
                nc.tensor.matmul(
                    pv[:],
                    lhsT=xt[kt][:, t * P:(t + 1) * P],
                    rhs=wv_s[:, kt, :],
                    start=kt == 0,
                    stop=kt == KT4 - 1,
                )
            nc.vector.tensor_tensor(
                Vb[:, t, :, 0:D],
                pv.rearrange("p (h d) -> p h d", d=D),
                bvr_s[:],
                add,
            )

        def emit_attn(hp, c, with_v=False, fill=(), drain_prev=None):
            """fill: closures emitting extra PE work, consumed one per t-slot
            starting at the back; drain_prev: previous block's deferred drain,
            emitted at t==2 so it never head-of-line-blocks this block's score
            matmuls."""
            hA, hB = 2 * hp, 2 * hp + 1
            jt = hp  # feature tile holding this head pair
            cs = slice(c * 512, (c + 1) * 512)
            # one 2-bank PSUM tile: head A accumulates in cols 0:512, head B
            # in cols 512:1024; row 64 of each half is the softmax denominator
            pvM = ps.tile([D + 1, 1024], dt, tag="pv", bufs=1, name=f"pvM{hp}{c}")
            fill = dict(fill)  # t-slot -> closure, placed just-in-time
            for t in range(16):
                # one PSUM tile holds the head pair's scores for key-tile t:
                # head A on PE rows 0:64 -> cols 0:512, head B on rows 64:128
                # -> cols 512:1024, emitted adjacently for row-tile concurrency
                sM = ps.tile([P, 1024], dt, tag="s", name=f"sM{hp}{c}{t}")
                nc.tensor.matmul(
                    sM[:, 0:512],
                    lhsT=KTt[0:D, jt, t * P:(t + 1) * P],
                    rhs=QT[0:D, jt, cs],
                    start=True,
                    stop=True,
                )
                nc.tensor.matmul(
                    sM[:, 512:1024],
                    lhsT=KTt[D:P, jt, t * P:(t + 1) * P],
                    rhs=QT[D:P, jt, cs],
                    start=True,
                    stop=True,
                )
                ptM = ptp.tile([P, 1024], bf16, tag="pt", name=f"pt{hp}{c}{t}")
                nc.scalar.activation(ptM[:], sM[:], Exp, scale=SCALE)
                if with_v:
                    emit_v_tile(t)
                if t == 2 and drain_prev is not None:
                    drain_prev()
                if t in fill:
                    fill.pop(t)()
                nc.tensor.matmul(
                    pvM[:, 0:512],
                    lhsT=Vb[:, t, hA, 0:D + 1],
                    rhs=ptM[:, 0:512],
                    start=t == 0,
                    stop=t == 15,
                )
                nc.tensor.matmul(
                    pvM[:, 512:1024],
                    lhsT=Vb[:, t, hB, 0:D + 1],
                    rhs=ptM[:, 512:1024],
                    start=t == 0,
                    stop=t == 15,
                )
            assert not fill

            def drain():
                # denominators live on PSUM row 64 (cols 0:512 head A,
                # 512:1024 head B).  Broadcast the RAW denominators to the 128
                # feature rows via two 1-deep PE matmuls against constant 0/1
                # selectors, take the reciprocal of the (base-0) broadcast,
                # then scale each head's copied-out rows.
                rb = nrm.tile([1, 1024], bf16, tag="rb", name=f"rb{hp}{c}")
                nc.vector.tensor_copy(rb[:], pvM[D:D + 1, :])
                pr = ps.tile([P, 512], dt, tag="w", name=f"pr{hp}{c}")
                nc.tensor.matmul(pr[:], lhsT=selA_s[:], rhs=rb[0:1, 0:512],
                                 start=True, stop=False)
                nc.tensor.matmul(pr[:], lhsT=selB_s[:], rhs=rb[0:1, 512:1024],
                                 start=False, stop=True)
                rec = nrm.tile([P, 512], dt, tag="rec", name=f"rec{hp}{c}")
                nc.vector.reciprocal_approx_fast(rec[:], pr[:])
                # copy each head's 64 output rows to SBUF, then scale (TT
                # allows one PSUM input and needs matching partition bases)
                nc.vector.tensor_copy(outT[0:D, hp, cs], pvM[0:D, 0:512])
                nc.vector.tensor_tensor(
                    outT[0:D, hp, cs], outT[0:D, hp, cs], rec[0:D, :], mult
                )
                nc.vector.tensor_copy(outT[D:P, hp, cs], pvM[0:D, 512:1024])
                nc.vector.tensor_tensor(
                    outT[D:P, hp, cs], outT[D:P, hp, cs], rec[D:P, :], mult
                )
            return drain

        def o_tile(m):
            def emit():
                py = ps.tile([P, 512], dt, tag="w", name=f"py{m}")
                for et in range(KT4):
                    nc.tensor.matmul(
                        py[:],
                        lhsT=outT[:, et, m * P:(m + 1) * P],
                        rhs=wo_s[:, et, :],
                        start=et == 0,
                        stop=et == KT4 - 1,
                    )
                yt = yp.tile([P, 512], dt, tag="y", name=f"yt{m}")
                nc.vector.tensor_tensor(yt[:], py[:], bor_s[:], add)
                nc.sync.dma_start(y_d[m * P:(m + 1) * P, :], yt[:])
            return emit

        # Prerequisites for block (1,0) that can't ride a fill slot: Q(1,c=0)
        # and K(1,c=0), emitted right after the startup drains (all xt/weights
        # are resident by then; these overlap block (0,0)'s exp stream).
        q_tile(1, 0)()
        k_tile(1, 0)()

        # chunk-outer: chunk 0's output projection + DMA overlap chunk 1's
        # ScalarE-bound attention.  Projections and chunk-0 output tiles are
        # placed just-in-time in later blocks' t-slots (the PE has slack while
        # ScalarE works through the exp stream), and each block's drain is
        # deferred into the next block so its recip chain never head-of-line-
        # blocks the score matmuls.  Block (0,0) carries the 16 V-projection
        # tiles and no other fills.
        fills = {
            (1, 0): {2: k_tile(1, 1), 5: k_tile(1, 2), 8: k_tile(1, 3),
                     11: q_tile(2, 0), 14: k_tile(2, 0)},
            (2, 0): {2: k_tile(2, 1), 5: k_tile(2, 2), 8: k_tile(2, 3),
                     11: q_tile(3, 0), 14: k_tile(3, 0)},
            (3, 0): {2: k_tile(3, 1), 5: k_tile(3, 2), 8: k_tile(3, 3),
                     11: q_tile(1, 1)},
            (0, 1): {5: q_tile(2, 1)},
            (1, 1): {3: o_tile(0), 5: q_tile(3, 1), 8: o_tile(1),
                     11: o_tile(2)},
            (2, 1): {3: o_tile(3)},
        }
        drain = None
        for c in range(2):
            for hp in range(4):
                drain = emit_attn(hp, c, with_v=(c == 0 and hp == 0),
                                  fill=fills.get((hp, c), {}),
                                  drain_prev=drain)
        drain()
        for m in range(4, 8):
            o_tile(m)()

        if debug:
            nc.sync.dma_start(dQT.ap(), QT[:])
            nc.sync.dma_start(dKT.ap(), KTt[:])
            nc.sync.dma_start(dVb.ap(), Vb[:])
            nc.sync.dma_start(doutT.ap(), outT[:])

    nc.finalize()
    return nc


def _get_program(debug=False):
    key = ("nc", debug)
    if key not in _CACHE:
        _CACHE[key] = _build_program(debug)
    return _CACHE[key]


def _host_inputs(x, Wq, bq, Wk, bk, Wv, bv, Wo, bo):
    import ml_dtypes
    f32 = np.float32
    bf = ml_dtypes.bfloat16
    wqT = np.ascontiguousarray(np.asarray(Wq, f32).T).reshape(KT4, P, EMB).astype(bf)
    wkT = np.ascontiguousarray(np.asarray(Wk, f32).T).reshape(KT4, P, EMB).astype(bf)
    wvT = np.ascontiguousarray(np.asarray(Wv, f32).T).reshape(KT4, P, EMB).astype(bf)
    woT = np.ascontiguousarray(np.asarray(Wo, f32).T).reshape(KT4, P, EMB).astype(bf)
    bq2 = np.ascontiguousarray(np.asarray(bq, f32).reshape(KT4, P).T)
    bk2 = np.ascontiguousarray(np.asarray(bk, f32).reshape(KT4, P).T)
    bvr = np.ascontiguousarray(np.tile(np.asarray(bv, f32), (P, 1)))
    bor = np.ascontiguousarray(np.tile(np.asarray(bo, f32), (P, 1)))
    sel2 = np.zeros((2, P), f32)
    for m in range(P):
        sel2[m // D, m] = 1.0
    sel2 = sel2.astype(bf)

    shared = dict(wq=wqT, wk=wkT, wv=wvT, wo=woT, bq2=bq2, bk2=bk2,
                  bvr=bvr, bor=bor, sel2=sel2)
    x = np.asarray(x, f32)
    in_maps = []
    for c in range(NCORES):
        b, hf = c // 2, c % 2
        xb = x[b]
        # queries first; key order is irrelevant as long as K and V agree
        xr = np.concatenate(
            [xb[hf * HALF:(hf + 1) * HALF], xb[(1 - hf) * HALF:(2 - hf) * HALF]], 0
        )
        xT = np.ascontiguousarray(xr.T).reshape(KT4, P, N).astype(bf)
        in_maps.append(dict(shared, xT=xT))
    return in_maps


def kernel(x, Wq, bq, Wk, bk, Wv, bv, Wo, bo, _trace=False, _trace_cores=None,
           _debug=False):
    from concourse.bass_utils import run_bass_kernel_spmd

    nc = _get_program(_debug)
    in_maps = _host_inputs(x, Wq, bq, Wk, bk, Wv, bv, Wo, bo)
    res = run_bass_kernel_spmd(
        nc, in_maps, list(range(NCORES)), trace=_trace,
        trace_cores=_trace_cores,
    )
    out = np.empty((B, N, EMB), np.float32)
    for c in range(NCORES):
        b, hf = c // 2, c % 2
        out[b, hf * HALF:(hf + 1) * HALF] = res.results[c]["y"]
    if _trace:
        _CACHE["last_results"] = res
    return out


# revision 31
# speedup vs baseline: 1.2472x; 1.0134x over previous
"""Multi-head attention (EMB=512, HEADS=8, x:(4,2048,512)) on 8 Trainium2 cores.

Sharding: zero-collective split — core c handles batch c//2, query rows
(c%2)*1024..(c%2+1)*1024, ALL heads.  K/V projections for the full batch are
computed redundantly on the 2 cores sharing a batch (no collectives at all).

All matmul operands are bf16: on TRN2 a 128-deep-contraction fp32r matmul
runs 2 cycles/row ("HIGH" replicated mode) while bf16 runs 1 cycle/row with
FWL weight loads, so bf16 halves projection time and halves input DMA bytes.

Device-side dataflow (per core, everything SBUF-resident):
  xT (host-transposed)           [512, 2048]   keys reordered so queries first
  Q^T = WqT.T @ xT  (+bq)        [512, 1024]   feature-major (bf16)
  K^T = WkT.T @ xT  (+bk)        [512, 2048]   feature-major (bf16)
  V~  = xT.T @ WvT  (+bv, ones)  [2048, 8*72]  token-major, per-head ones col
  S^T(t) = K^T_h.T @ Q^T_h       one [128,1024] PSUM tile per key-tile t holds
                                 BOTH heads of the pair: cols 0:512 head A on
                                 PE rows 0:64, cols 512:1024 head B on rows
                                 64:128 — emitted adjacently so the pair runs
                                 concurrently via PE row tiling
  P^T = exp(S^T / sqrt(512))     (ScalarE, fused drain from PSUM -> bf16)
  outT~ = V~_h.T @ P^T           [72, 512]     row 64+h = softmax denominator
  outT = outT~ * bcast(1/denom)  normalization fused into the PSUM drain
  y = outT.T @ WoT (+bo)         [1024, 512]   token-major, DMA out

Loop order: query-chunk c is OUTER so chunk 0's output projection + DMA
overlap chunk 1's (ScalarE-bound) attention.  Startup is kt-outer so the
first Q/K projections overlap the input DMA stream.  V-projection tiles are
emitted inside the first attention block's t-loop, just in time for PV.
"""

import sys
import os

for _p in ("/opt/trn_rl_repo", "/root/.axon_site/_ro/trn_rl_repo"):
    if os.path.isdir(_p) and _p not in sys.path:
        sys.path.append(_p)

import numpy as np

EMB = 512
HEADS = 8
D = 64  # head dim
B = 4
N = 2048  # keys / tokens per batch
HALF = 1024  # queries per core
P = 128
NCORES = 8
KT4 = EMB // P  # 4 contraction tiles
SCALE = float(1.0 / np.sqrt(np.float32(EMB)))

_CACHE = {}


def _build_program(debug=False):
    from concourse import bacc
    import concourse.mybir as mybir
    import concourse.tile as tile
    from contextlib import ExitStack

    dt = mybir.dt.float32
    bf16 = mybir.dt.bfloat16
    nc = bacc.Bacc("TRN2", target_bir_lowering=False)

    xT_d = nc.dram_tensor("xT", [KT4, P, N], bf16, kind="ExternalInput")
    wq_d = nc.dram_tensor("wq", [KT4, P, EMB], bf16, kind="ExternalInput")
    wk_d = nc.dram_tensor("wk", [KT4, P, EMB], bf16, kind="ExternalInput")
    wv_d = nc.dram_tensor("wv", [KT4, P, EMB], bf16, kind="ExternalInput")
    wo_d = nc.dram_tensor("wo", [KT4, P, EMB], bf16, kind="ExternalInput")
    bq_d = nc.dram_tensor("bq2", [P, KT4], dt, kind="ExternalInput")
    bk_d = nc.dram_tensor("bk2", [P, KT4], dt, kind="ExternalInput")
    bvr_d = nc.dram_tensor("bvr", [P, EMB], dt, kind="ExternalInput")
    bor_d = nc.dram_tensor("bor", [P, EMB], dt, kind="ExternalInput")
    sel_d = nc.dram_tensor("sel2", [2, P], bf16, kind="ExternalInput")  # row 0: p<64, row 1: p>=64
    y_d = nc.dram_tensor("y", [HALF, EMB], dt, kind="ExternalOutput")
    if debug:
        dQT = nc.dram_tensor("dQT", [P, KT4, HALF], bf16, kind="ExternalOutput")
        dKT = nc.dram_tensor("dKT", [P, KT4, N], bf16, kind="ExternalOutput")
        dVb = nc.dram_tensor("dVb", [P, 16, HEADS, D + 8], bf16, kind="ExternalOutput")
        doutT = nc.dram_tensor("doutT", [P, KT4, HALF], bf16, kind="ExternalOutput")

    Exp = mybir.ActivationFunctionType.Exp
    mult = mybir.AluOpType.mult
    add = mybir.AluOpType.add

    with tile.TileContext(nc) as tc, ExitStack() as ctx:
        big = ctx.enter_context(tc.tile_pool(name="big", bufs=4))
        ptp = ctx.enter_context(tc.tile_pool(name="ptp", bufs=12))
        wp = ctx.enter_context(tc.tile_pool(name="wp", bufs=1))
        pers = ctx.enter_context(tc.tile_pool(name="pers", bufs=1))
        yp = ctx.enter_context(tc.tile_pool(name="yp", bufs=2))
        nrm = ctx.enter_context(tc.tile_pool(name="nrm", bufs=2))
        # PSUM: tag "s" 2 x [128,1024] score slots (4 banks) + tag "pv" 1 x
        # [65,1024] (2 banks) + tag "w" 2 x [128,512] (2 banks) for the
        # projection/broadcast tiles, so they never disturb the score-tile
        # rotation that paces ScalarE
        ps = ctx.enter_context(tc.tile_pool(name="ps", bufs=2, space="PSUM"))

        # ---- input loads, ordered to match first-use ----
        # tiny bias/selector tensors first (the startup drains need them),
        # then round kt: wq[kt], wk[kt], xt[kt] feeding the kt-outer startup
        xt = []
        wq_s = wp.tile([P, KT4, EMB], bf16, name="wqs", tag="wqs")
        wk_s = wp.tile([P, KT4, EMB], bf16, name="wks", tag="wks")
        wv_s = wp.tile([P, KT4, EMB], bf16, name="wvs", tag="wvs")
        wo_s = wp.tile([P, KT4, EMB], bf16, name="wos", tag="wos")
        bq_s = pers.tile([P, KT4], dt, name="bqs")
        bk_s = pers.tile([P, KT4], dt, name="bks")
        selA_s = pers.tile([1, P], bf16, name="selAs")
        selB_s = pers.tile([1, P], bf16, name="selBs")
        for kt in range(KT4):
            nc.sync.dma_start(wq_s[:, kt], wq_d[kt])
            nc.sync.dma_start(wk_s[:, kt], wk_d[kt])
            t = big.tile([P, N], bf16, name=f"xt{kt}", tag="big")
            nc.sync.dma_start(t[:], xT_d[kt])
            xt.append(t)
            if kt == 0:
                # small strided bias/selector loads ride after the first round
                # (needed only by the startup drains at ~20us)
                nc.sync.dma_start(bq_s[:], bq_d[:])
                nc.sync.dma_start(bk_s[:], bk_d[:])
                nc.sync.dma_start(selA_s[:], sel_d[0:1])
                nc.sync.dma_start(selB_s[:], sel_d[1:2])
        for kt in range(KT4):
            nc.sync.dma_start(wv_s[:, kt], wv_d[kt])
        bvr_s = pers.tile([P, HEADS, D], dt, name="bvrs")
        nc.sync.dma_start(bvr_s[:], bvr_d.ap().rearrange("p (h d) -> p h d", d=D))
        for kt in range(KT4):
            nc.sync.dma_start(wo_s[:, kt], wo_d[kt])
        bor_s = pers.tile([P, EMB], dt, name="bors")
        nc.sync.dma_start(bor_s[:], bor_d[:])

        # ---- persistent intermediates ----
        QT = pers.tile([P, KT4, HALF], bf16, name="QT")
        KTt = pers.tile([P, KT4, N], bf16, name="KTt")
        Vb = pers.tile([P, 16, HEADS, D + 8], bf16, name="Vb")
        outT = pers.tile([P, KT4, HALF], bf16, name="outT")

        # single ones column per head: PV lands the softmax denominator on
        # PSUM partition 64 (aligned for the base-shifting drain copy)
        nc.vector.memset(Vb[:, :, :, D:D + 8], 0.0)
        nc.vector.memset(Vb[:, :, :, D], 1.0)

        # ---- startup: Q(0) and K(0) kt-outer so PE starts on first DMAs ----
        sQ = ps.tile([P, 1024], dt, tag="s", name="sQ0")
        sK1 = ps.tile([P, 1024], dt, tag="s", name="sK01")
        sK2 = ps.tile([P, 512], dt, tag="w", name="sK2")
        sK3 = ps.tile([P, 512], dt, tag="w", name="sK3")
        for kt in range(KT4):
            for c in range(2):
                nc.tensor.matmul(
                    sQ[:, c * 512:(c + 1) * 512],
                    lhsT=wq_s[:, kt, 0:P],
                    rhs=xt[kt][:, c * 512:(c + 1) * 512],
                    start=kt == 0,
                    stop=kt == KT4 - 1,
                )
            for c in range(4):
                sK = (sK1[:, 0:512], sK1[:, 512:1024], sK2[:], sK3[:])[c]
                nc.tensor.matmul(
                    sK,
                    lhsT=wk_s[:, kt, 0:P],
                    rhs=xt[kt][:, c * 512:(c + 1) * 512],
                    start=kt == 0,
                    stop=kt == KT4 - 1,
                )
        for c in range(2):
            nc.vector.tensor_scalar_add(
                QT[:, 0, c * 512:(c + 1) * 512],
                sQ[:, c * 512:(c + 1) * 512], bq_s[:, 0:1],
            )
        for c in range(4):
            sK = (sK1[:, 0:512], sK1[:, 512:1024], sK2[:], sK3[:])[c]
            nc.vector.tensor_scalar_add(
                KTt[:, 0, c * 512:(c + 1) * 512], sK, bk_s[:, 0:1],
            )

        def q_tile(jt, c):
            def emit():
                pq = ps.tile([P, 512], dt, tag="w", name=f"pq{jt}{c}")
                for kt in range(KT4):
                    nc.tensor.matmul(
                        pq[:],
                        lhsT=wq_s[:, kt, jt * P:(jt + 1) * P],
                        rhs=xt[kt][:, c * 512:(c + 1) * 512],
                        start=kt == 0,
                        stop=kt == KT4 - 1,
                    )
                nc.vector.tensor_scalar_add(
                    QT[:, jt, c * 512:(c + 1) * 512], pq[:], bq_s[:, jt:jt + 1]
                )
            return emit

        def k_tile(jt, c):
            def emit():
                pk = ps.tile([P, 512], dt, tag="w", name=f"pk{jt}{c}")
                for kt in range(KT4):
                    nc.tensor.matmul(
                        pk[:],
                        lhsT=wk_s[:, kt, jt * P:(jt + 1) * P],
                        rhs=xt[kt][:, c * 512:(c + 1) * 512],
                        start=kt == 0,
                        stop=kt == KT4 - 1,
                    )
                nc.vector.tensor_scalar_add(
                    KTt[:, jt, c * 512:(c + 1) * 512], pk[:], bk_s[:, jt:jt + 1]
                )
            return emit

        def qk_fill(jt):
            return [q_tile(jt, 0), q_tile(jt, 1)] + [k_tile(jt, c) for c in range(4)]

        def emit_v_tile(t):
            pv = ps.tile([P, 512], dt, tag="w", name=f"pvv{t}")
            for kt in range(KT4):
                nc.tensor.matmul(
                    pv[:],
                    lhsT=xt[kt][:, t * P:(t + 1) * P],
                    rhs=wv_s[:, kt, :],
                    start=kt == 0,
                    stop=kt == KT4 - 1,
                )
            nc.vector.tensor_tensor(
                Vb[:, t, :, 0:D],
                pv.rearrange("p (h d) -> p h d", d=D),
                bvr_s[:],
                add,
            )

        def emit_attn(hp, c, with_v=False, fill=(), drain_prev=None):
            """fill: closures emitting extra PE work, consumed one per t-slot
            starting at the back; drain_prev: previous block's deferred drain,
            emitted at t==2 so it never head-of-line-blocks this block's score
            matmuls."""
            hA, hB = 2 * hp, 2 * hp + 1
            jt = hp  # feature tile holding this head pair
            cs = slice(c * 512, (c + 1) * 512)
            # one 2-bank PSUM tile: head A accumulates in cols 0:512, head B
            # in cols 512:1024; row 64 of each half is the softmax denominator
            pvM = ps.tile([D + 1, 1024], dt, tag="pv", bufs=1, name=f"pvM{hp}{c}")
            fill = dict(fill)  # t-slot -> closure, placed just-in-time
            for t in range(16):
                # one PSUM tile holds the head pair's scores for key-tile t:
                # head A on PE rows 0:64 -> cols 0:512, head B on rows 64:128
                # -> cols 512:1024, emitted adjacently for row-tile concurrency
                sM = ps.tile([P, 1024], dt, tag="s", name=f"sM{hp}{c}{t}")
                nc.tensor.matmul(
                    sM[:, 0:512],
                    lhsT=KTt[0:D, jt, t * P:(t + 1) * P],
                    rhs=QT[0:D, jt, cs],
                    start=True,
                    stop=True,
                )
                nc.tensor.matmul(
                    sM[:, 512:1024],
                    lhsT=KTt[D:P, jt, t * P:(t + 1) * P],
                    rhs=QT[D:P, jt, cs],
                    start=True,
                    stop=True,
                )
                ptM = ptp.tile([P, 1024], bf16, tag="pt", name=f"pt{hp}{c}{t}")
                nc.scalar.activation(ptM[:], sM[:], Exp, scale=SCALE)
                if with_v:
                    emit_v_tile(t)
                if t == 2 and drain_prev is not None:
                    drain_prev()
                if t in fill:
                    fill.pop(t)()
                nc.tensor.matmul(
                    pvM[:, 0:512],
                    lhsT=Vb[:, t, hA, 0:D + 1],
                    rhs=ptM[:, 0:512],
                    start=t == 0,
                    stop=t == 15,
                )
                nc.tensor.matmul(
                    pvM[:, 512:1024],
                    lhsT=Vb[:, t, hB, 0:D + 1],
                    rhs=ptM[:, 512:1024],
                    start=t == 0,
                    stop=t == 15,
                )
            assert not fill

            def drain():
                # denominators live on PSUM row 64 (cols 0:512 head A,
                # 512:1024 head B).  Broadcast the RAW denominators to the 128
                # feature rows via two 1-deep PE matmuls against constant 0/1
                # selectors, take the reciprocal of the (base-0) broadcast,
                # then scale each head's copied-out rows.
                rb = nrm.tile([1, 1024], bf16, tag="rb", name=f"rb{hp}{c}")
                nc.vector.tensor_copy(rb[:], pvM[D:D + 1, :])
                pr = ps.tile([P, 512], dt, tag="w", name=f"pr{hp}{c}")
                nc.tensor.matmul(pr[:], lhsT=selA_s[:], rhs=rb[0:1, 0:512],
                                 start=True, stop=False)
                nc.tensor.matmul(pr[:], lhsT=selB_s[:], rhs=rb[0:1, 512:1024],
                                 start=False, stop=True)
                rec = nrm.tile([P, 512], dt, tag="rec", name=f"rec{hp}{c}")
                nc.vector.reciprocal_approx_fast(rec[:], pr[:])
                # copy each head's 64 output rows to SBUF, then scale (TT
                # allows one PSUM input and needs matching partition bases)
                nc.vector.tensor_copy(outT[0:D, hp, cs], pvM[0:D, 0:512])
                nc.vector.tensor_tensor(
                    outT[0:D, hp, cs], outT[0:D, hp, cs], rec[0:D, :], mult
                )
                nc.vector.tensor_copy(outT[D:P, hp, cs], pvM[0:D, 512:1024])
                nc.vector.tensor_tensor(
                    outT[D:P, hp, cs], outT[D:P, hp, cs], rec[D:P, :], mult
                )
            return drain

        def o_tile(m):
            def emit():
                py = ps.tile([P, 512], dt, tag="w", name=f"py{m}")
                for et in range(KT4):
                    nc.tensor.matmul(
                        py[:],
                        lhsT=outT[:, et, m * P:(m + 1) * P],
                        rhs=wo_s[:, et, :],
                        start=et == 0,
                        stop=et == KT4 - 1,
                    )
                yt = yp.tile([P, 512], dt, tag="y", name=f"yt{m}")
                nc.vector.tensor_tensor(yt[:], py[:], bor_s[:], add)
                nc.sync.dma_start(y_d[m * P:(m + 1) * P, :], yt[:])
            return emit

        # Prerequisites for block (1,0) that can't ride a fill slot: Q(1,c=0)
        # and K(1,c=0), emitted right after the startup drains (all xt/weights
        # are resident by then; these overlap block (0,0)'s exp stream).
        q_tile(1, 0)()
        k_tile(1, 0)()

        # chunk-outer: chunk 0's output projection + DMA overlap chunk 1's
        # ScalarE-bound attention.  Projections and chunk-0 output tiles are
        # placed just-in-time in later blocks' t-slots (the PE has slack while
        # ScalarE works through the exp stream), and each block's drain is
        # deferred into the next block so its recip chain never head-of-line-
        # blocks the score matmuls.  Block (0,0) carries the 16 V-projection
        # tiles and no other fills.
        fills = {
            (1, 0): {1: k_tile(1, 1), 3: k_tile(1, 2), 5: k_tile(1, 3),
                     7: q_tile(2, 0), 9: k_tile(2, 0)},
            (2, 0): {1: k_tile(2, 1), 3: k_tile(2, 2), 5: k_tile(2, 3),
                     7: q_tile(3, 0), 9: k_tile(3, 0)},
            (3, 0): {1: k_tile(3, 1), 3: k_tile(3, 2), 5: k_tile(3, 3),
                     7: q_tile(1, 1)},
            (0, 1): {5: q_tile(2, 1)},
            (1, 1): {3: o_tile(0), 5: q_tile(3, 1), 8: o_tile(1),
                     11: o_tile(2)},
            (2, 1): {3: o_tile(3)},
        }
        drain = None
        for c in range(2):
            for hp in range(4):
                drain = emit_attn(hp, c, with_v=(c == 0 and hp == 0),
                                  fill=fills.get((hp, c), {}),
                                  drain_prev=drain)
        drain()
        for m in range(4, 8):
            o_tile(m)()

        if debug:
            nc.sync.dma_start(dQT.ap(), QT[:])
            nc.sync.dma_start(dKT.ap(), KTt[:])
            nc.sync.dma_start(dVb.ap(), Vb[:])
            nc.sync.dma_start(doutT.ap(), outT[:])

    nc.finalize()
    return nc


def _get_program(debug=False):
    key = ("nc", debug)
    if key not in _CACHE:
        _CACHE[key] = _build_program(debug)
    return _CACHE[key]


def _host_inputs(x, Wq, bq, Wk, bk, Wv, bv, Wo, bo):
    import ml_dtypes
    f32 = np.float32
    bf = ml_dtypes.bfloat16
    wqT = np.ascontiguousarray(np.asarray(Wq, f32).T).reshape(KT4, P, EMB).astype(bf)
    wkT = np.ascontiguousarray(np.asarray(Wk, f32).T).reshape(KT4, P, EMB).astype(bf)
    wvT = np.ascontiguousarray(np.asarray(Wv, f32).T).reshape(KT4, P, EMB).astype(bf)
    woT = np.ascontiguousarray(np.asarray(Wo, f32).T).reshape(KT4, P, EMB).astype(bf)
    bq2 = np.ascontiguousarray(np.asarray(bq, f32).reshape(KT4, P).T)
    bk2 = np.ascontiguousarray(np.asarray(bk, f32).reshape(KT4, P).T)
    bvr = np.ascontiguousarray(np.tile(np.asarray(bv, f32), (P, 1)))
    bor = np.ascontiguousarray(np.tile(np.asarray(bo, f32), (P, 1)))
    sel2 = np.zeros((2, P), f32)
    for m in range(P):
        sel2[m // D, m] = 1.0
    sel2 = sel2.astype(bf)

    shared = dict(wq=wqT, wk=wkT, wv=wvT, wo=woT, bq2=bq2, bk2=bk2,
                  bvr=bvr, bor=bor, sel2=sel2)
    x = np.asarray(x, f32)
    in_maps = []
    for c in range(NCORES):
        b, hf = c // 2, c % 2
        xb = x[b]
        # queries first; key order is irrelevant as long as K and V agree
        xr = np.concatenate(
            [xb[hf * HALF:(hf + 1) * HALF], xb[(1 - hf) * HALF:(2 - hf) * HALF]], 0
        )
        xT = np.ascontiguousarray(xr.T).reshape(KT4, P, N).astype(bf)
        in_maps.append(dict(shared, xT=xT))
    return in_maps


def kernel(x, Wq, bq, Wk, bk, Wv, bv, Wo, bo, _trace=False, _trace_cores=None,
           _debug=False):
    from concourse.bass_utils import run_bass_kernel_spmd

    nc = _get_program(_debug)
    in_maps = _host_inputs(x, Wq, bq, Wk, bk, Wv, bv, Wo, bo)
    res = run_bass_kernel_spmd(
        nc, in_maps, list(range(NCORES)), trace=_trace,
        trace_cores=_trace_cores,
    )
    out = np.empty((B, N, EMB), np.float32)
    for c in range(NCORES):
        b, hf = c // 2, c % 2
        out[b, hf * HALF:(hf + 1) * HALF] = res.results[c]["y"]
    if _trace:
        _CACHE["last_results"] = res
    return out


# revision 35
# speedup vs baseline: 1.2603x; 1.0105x over previous
"""Multi-head attention (EMB=512, HEADS=8, x:(4,2048,512)) on 8 Trainium2 cores.

Sharding: zero-collective split — core c handles batch c//2, query rows
(c%2)*1024..(c%2+1)*1024, ALL heads.  K/V projections for the full batch are
computed redundantly on the 2 cores sharing a batch (no collectives at all).

All matmul operands are bf16: on TRN2 a 128-deep-contraction fp32r matmul
runs 2 cycles/row ("HIGH" replicated mode) while bf16 runs 1 cycle/row with
FWL weight loads, so bf16 halves projection time and halves input DMA bytes.

Device-side dataflow (per core, everything SBUF-resident):
  xT (host-transposed)           [512, 2048]   keys reordered so queries first
  Q^T = WqT.T @ xT  (+bq)        [512, 1024]   feature-major (bf16)
  K^T = WkT.T @ xT  (+bk)        [512, 2048]   feature-major (bf16)
  V~  = xT.T @ WvT  (+bv, ones)  [2048, 8*72]  token-major, per-head ones col
  S^T(t) = K^T_h.T @ Q^T_h       one [128,1024] PSUM tile per key-tile t holds
                                 BOTH heads of the pair: cols 0:512 head A on
                                 PE rows 0:64, cols 512:1024 head B on rows
                                 64:128 — emitted adjacently so the pair runs
                                 concurrently via PE row tiling
  P^T = exp(S^T / sqrt(512))     (ScalarE, fused drain from PSUM -> bf16)
  outT~ = V~_h.T @ P^T           [72, 512]     row 64+h = softmax denominator
  outT = outT~ * bcast(1/denom)  normalization fused into the PSUM drain
  y = outT.T @ WoT (+bo)         [1024, 512]   token-major, DMA out

Loop order: query-chunk c is OUTER so chunk 0's output projection + DMA
overlap chunk 1's (ScalarE-bound) attention.  Startup is kt-outer so the
first Q/K projections overlap the input DMA stream.  V-projection tiles are
emitted inside the first attention block's t-loop, just in time for PV.
"""

import sys
import os

for _p in ("/opt/trn_rl_repo", "/root/.axon_site/_ro/trn_rl_repo"):
    if os.path.isdir(_p) and _p not in sys.path:
        sys.path.append(_p)

import numpy as np

EMB = 512
HEADS = 8
D = 64  # head dim
B = 4
N = 2048  # keys / tokens per batch
HALF = 1024  # queries per core
P = 128
NCORES = 8
KT4 = EMB // P  # 4 contraction tiles
SCALE = float(1.0 / np.sqrt(np.float32(EMB)))

_CACHE = {}


def _build_program(debug=False):
    from concourse import bacc
    import concourse.mybir as mybir
    import concourse.tile as tile
    from contextlib import ExitStack

    dt = mybir.dt.float32
    bf16 = mybir.dt.bfloat16
    nc = bacc.Bacc("TRN2", target_bir_lowering=False)

    xT_d = nc.dram_tensor("xT", [KT4, P, N], bf16, kind="ExternalInput")
    wq_d = nc.dram_tensor("wq", [KT4, P, EMB], bf16, kind="ExternalInput")
    wk_d = nc.dram_tensor("wk", [KT4, P, EMB], bf16, kind="ExternalInput")
    wv_d = nc.dram_tensor("wv", [KT4, P, EMB], bf16, kind="ExternalInput")
    wo_d = nc.dram_tensor("wo", [KT4, P, EMB], bf16, kind="ExternalInput")
    bq_d = nc.dram_tensor("bq2", [P, KT4], dt, kind="ExternalInput")
    bk_d = nc.dram_tensor("bk2", [P, KT4], dt, kind="ExternalInput")
    bvr_d = nc.dram_tensor("bvr", [P, EMB], dt, kind="ExternalInput")
    bor_d = nc.dram_tensor("bor", [P, EMB], dt, kind="ExternalInput")
    sel_d = nc.dram_tensor("sel2", [2, P], bf16, kind="ExternalInput")  # row 0: p<64, row 1: p>=64
    y_d = nc.dram_tensor("y", [HALF, EMB], dt, kind="ExternalOutput")
    if debug:
        dQT = nc.dram_tensor("dQT", [P, KT4, HALF], bf16, kind="ExternalOutput")
        dKT = nc.dram_tensor("dKT", [P, KT4, N], bf16, kind="ExternalOutput")
        dVb = nc.dram_tensor("dVb", [P, 16, HEADS, D + 8], bf16, kind="ExternalOutput")
        doutT = nc.dram_tensor("doutT", [P, KT4, HALF], bf16, kind="ExternalOutput")

    Exp = mybir.ActivationFunctionType.Exp
    mult = mybir.AluOpType.mult
    add = mybir.AluOpType.add

    with tile.TileContext(nc) as tc, ExitStack() as ctx:
        big = ctx.enter_context(tc.tile_pool(name="big", bufs=4))
        ptp = ctx.enter_context(tc.tile_pool(name="ptp", bufs=12))
        wp = ctx.enter_context(tc.tile_pool(name="wp", bufs=1))
        pers = ctx.enter_context(tc.tile_pool(name="pers", bufs=1))
        yp = ctx.enter_context(tc.tile_pool(name="yp", bufs=2))
        nrm = ctx.enter_context(tc.tile_pool(name="nrm", bufs=2))
        # PSUM: tag "s" 2 x [128,1024] score slots (4 banks) + tag "pv" 1 x
        # [65,1024] (2 banks) + tag "w" 2 x [128,512] (2 banks) for the
        # projection/broadcast tiles, so they never disturb the score-tile
        # rotation that paces ScalarE
        ps = ctx.enter_context(tc.tile_pool(name="ps", bufs=2, space="PSUM"))

        # ---- input loads, ordered to match first-use ----
        # tiny bias/selector tensors first (the startup drains need them),
        # then round kt: wq[kt], wk[kt], xt[kt] feeding the kt-outer startup
        xt = []
        wq_s = wp.tile([P, KT4, EMB], bf16, name="wqs", tag="wqs")
        wk_s = wp.tile([P, KT4, EMB], bf16, name="wks", tag="wks")
        wv_s = wp.tile([P, KT4, EMB], bf16, name="wvs", tag="wvs")
        wo_s = wp.tile([P, KT4, EMB], bf16, name="wos", tag="wos")
        bq_s = pers.tile([P, KT4], dt, name="bqs")
        bk_s = pers.tile([P, KT4], dt, name="bks")
        selA_s = pers.tile([1, P], bf16, name="selAs")
        selB_s = pers.tile([1, P], bf16, name="selBs")
        for kt in range(KT4):
            # only head-pair-0 weight columns gate the startup matmuls
            nc.sync.dma_start(wq_s[:, kt, 0:P], wq_d[kt][:, 0:P])
            nc.sync.dma_start(wk_s[:, kt, 0:P], wk_d[kt][:, 0:P])
            t = big.tile([P, N], bf16, name=f"xt{kt}", tag="big")
            nc.sync.dma_start(t[:], xT_d[kt])
            xt.append(t)
            if kt == 0:
                # small strided bias/selector loads ride after the first round
                # (needed only by the startup drains at ~20us)
                nc.sync.dma_start(bq_s[:], bq_d[:])
                nc.sync.dma_start(bk_s[:], bk_d[:])
                nc.sync.dma_start(selA_s[:], sel_d[0:1])
                nc.sync.dma_start(selB_s[:], sel_d[1:2])
        for kt in range(KT4):
            nc.sync.dma_start(wv_s[:, kt], wv_d[kt])
        for kt in range(KT4):
            nc.sync.dma_start(wq_s[:, kt, P:EMB], wq_d[kt][:, P:EMB])
            nc.sync.dma_start(wk_s[:, kt, P:EMB], wk_d[kt][:, P:EMB])
        bvr_s = pers.tile([P, HEADS, D], dt, name="bvrs")
        nc.sync.dma_start(bvr_s[:], bvr_d.ap().rearrange("p (h d) -> p h d", d=D))
        for kt in range(KT4):
            nc.sync.dma_start(wo_s[:, kt], wo_d[kt])
        bor_s = pers.tile([P, EMB], dt, name="bors")
        nc.sync.dma_start(bor_s[:], bor_d[:])

        # ---- persistent intermediates ----
        QT = pers.tile([P, KT4, HALF], bf16, name="QT")
        KTt = pers.tile([P, KT4, N], bf16, name="KTt")
        Vb = pers.tile([P, 16, HEADS, D + 8], bf16, name="Vb")
        outT = pers.tile([P, KT4, HALF], bf16, name="outT")

        # single ones column per head: PV lands the softmax denominator on
        # PSUM partition 64 (aligned for the base-shifting drain copy)
        nc.vector.memset(Vb[:, :, :, D:D + 8], 0.0)
        nc.vector.memset(Vb[:, :, :, D], 1.0)

        # ---- startup: Q(0) and K(0) kt-outer so PE starts on first DMAs ----
        sQ = ps.tile([P, 1024], dt, tag="s", name="sQ0")
        sK1 = ps.tile([P, 1024], dt, tag="s", name="sK01")
        sK2 = ps.tile([P, 512], dt, tag="w", name="sK2")
        sK3 = ps.tile([P, 512], dt, tag="w", name="sK3")
        for kt in range(KT4):
            for c in range(2):
                nc.tensor.matmul(
                    sQ[:, c * 512:(c + 1) * 512],
                    lhsT=wq_s[:, kt, 0:P],
                    rhs=xt[kt][:, c * 512:(c + 1) * 512],
                    start=kt == 0,
                    stop=kt == KT4 - 1,
                )
            for c in range(4):
                sK = (sK1[:, 0:512], sK1[:, 512:1024], sK2[:], sK3[:])[c]
                nc.tensor.matmul(
                    sK,
                    lhsT=wk_s[:, kt, 0:P],
                    rhs=xt[kt][:, c * 512:(c + 1) * 512],
                    start=kt == 0,
                    stop=kt == KT4 - 1,
                )
        for c in range(2):
            nc.vector.tensor_scalar_add(
                QT[:, 0, c * 512:(c + 1) * 512],
                sQ[:, c * 512:(c + 1) * 512], bq_s[:, 0:1],
            )
        for c in range(4):
            sK = (sK1[:, 0:512], sK1[:, 512:1024], sK2[:], sK3[:])[c]
            nc.vector.tensor_scalar_add(
                KTt[:, 0, c * 512:(c + 1) * 512], sK, bk_s[:, 0:1],
            )

        def q_tile(jt, c):
            def emit():
                pq = ps.tile([P, 512], dt, tag="w", name=f"pq{jt}{c}")
                for kt in range(KT4):
                    nc.tensor.matmul(
                        pq[:],
                        lhsT=wq_s[:, kt, jt * P:(jt + 1) * P],
                        rhs=xt[kt][:, c * 512:(c + 1) * 512],
                        start=kt == 0,
                        stop=kt == KT4 - 1,
                    )
                nc.vector.tensor_scalar_add(
                    QT[:, jt, c * 512:(c + 1) * 512], pq[:], bq_s[:, jt:jt + 1]
                )
            return emit

        def k_tile(jt, c):
            def emit():
                pk = ps.tile([P, 512], dt, tag="w", name=f"pk{jt}{c}")
                for kt in range(KT4):
                    nc.tensor.matmul(
                        pk[:],
                        lhsT=wk_s[:, kt, jt * P:(jt + 1) * P],
                        rhs=xt[kt][:, c * 512:(c + 1) * 512],
                        start=kt == 0,
                        stop=kt == KT4 - 1,
                    )
                nc.vector.tensor_scalar_add(
                    KTt[:, jt, c * 512:(c + 1) * 512], pk[:], bk_s[:, jt:jt + 1]
                )
            return emit

        def qk_fill(jt):
            return [q_tile(jt, 0), q_tile(jt, 1)] + [k_tile(jt, c) for c in range(4)]

        def emit_v_tile(t):
            pv = ps.tile([P, 512], dt, tag="w", name=f"pvv{t}")
            for kt in range(KT4):
                nc.tensor.matmul(
                    pv[:],
                    lhsT=xt[kt][:, t * P:(t + 1) * P],
                    rhs=wv_s[:, kt, :],
                    start=kt == 0,
                    stop=kt == KT4 - 1,
                )
            nc.vector.tensor_tensor(
                Vb[:, t, :, 0:D],
                pv.rearrange("p (h d) -> p h d", d=D),
                bvr_s[:],
                add,
            )

        def emit_attn(hp, c, with_v=False, fill=(), drain_prev=None):
            """fill: closures emitting extra PE work, consumed one per t-slot
            starting at the back; drain_prev: previous block's deferred drain,
            emitted at t==2 so it never head-of-line-blocks this block's score
            matmuls."""
            hA, hB = 2 * hp, 2 * hp + 1
            jt = hp  # feature tile holding this head pair
            cs = slice(c * 512, (c + 1) * 512)
            # one 2-bank PSUM tile: head A accumulates in cols 0:512, head B
            # in cols 512:1024; row 64 of each half is the softmax denominator
            pvM = ps.tile([D + 1, 1024], dt, tag="pv", bufs=1, name=f"pvM{hp}{c}")
            fill = dict(fill)  # t-slot -> closure, placed just-in-time
            for t in range(16):
                # one PSUM tile holds the head pair's scores for key-tile t:
                # head A on PE rows 0:64 -> cols 0:512, head B on rows 64:128
                # -> cols 512:1024, emitted adjacently for row-tile concurrency
                sM = ps.tile([P, 1024], dt, tag="s", name=f"sM{hp}{c}{t}")
                nc.tensor.matmul(
                    sM[:, 0:512],
                    lhsT=KTt[0:D, jt, t * P:(t + 1) * P],
                    rhs=QT[0:D, jt, cs],
                    start=True,
                    stop=True,
                )
                nc.tensor.matmul(
                    sM[:, 512:1024],
                    lhsT=KTt[D:P, jt, t * P:(t + 1) * P],
                    rhs=QT[D:P, jt, cs],
                    start=True,
                    stop=True,
                )
                ptM = ptp.tile([P, 1024], bf16, tag="pt", name=f"pt{hp}{c}{t}")
                nc.scalar.activation(ptM[:], sM[:], Exp, scale=SCALE)
                if with_v:
                    emit_v_tile(t)
                if t == 2 and drain_prev is not None:
                    drain_prev()
                if t in fill:
                    fill.pop(t)()
                nc.tensor.matmul(
                    pvM[:, 0:512],
                    lhsT=Vb[:, t, hA, 0:D + 1],
                    rhs=ptM[:, 0:512],
                    start=t == 0,
                    stop=t == 15,
                )
                nc.tensor.matmul(
                    pvM[:, 512:1024],
                    lhsT=Vb[:, t, hB, 0:D + 1],
                    rhs=ptM[:, 512:1024],
                    start=t == 0,
                    stop=t == 15,
                )
            assert not fill

            def drain(last=False):
                # denominators live on PSUM row 64 (cols 0:512 head A,
                # 512:1024 head B).  Broadcast the RAW denominators to the 128
                # feature rows via two 1-deep PE matmuls against constant 0/1
                # selectors, take the reciprocal of the (base-0) broadcast,
                # then scale each head's copied-out rows.  For the LAST block
                # the copies run on the (now idle) ScalarE and pr moves to the
                # freed score slots, shortening the serial tail.
                Copy = mybir.ActivationFunctionType.Copy
                rb = nrm.tile([1, 1024], bf16, tag="rb", name=f"rb{hp}{c}")
                if last:
                    nc.scalar.activation(rb[:], pvM[D:D + 1, :], Copy)
                else:
                    nc.vector.tensor_copy(rb[:], pvM[D:D + 1, :])
                pr = ps.tile([P, 512], dt, tag="s" if last else "w",
                             name=f"pr{hp}{c}")
                nc.tensor.matmul(pr[:], lhsT=selA_s[:], rhs=rb[0:1, 0:512],
                                 start=True, stop=False)
                nc.tensor.matmul(pr[:], lhsT=selB_s[:], rhs=rb[0:1, 512:1024],
                                 start=False, stop=True)
                rec = nrm.tile([P, 512], dt, tag="rec", name=f"rec{hp}{c}")
                nc.vector.reciprocal_approx_fast(rec[:], pr[:])
                # copy each head's 64 output rows to SBUF, then scale (TT
                # allows one PSUM input and needs matching partition bases)
                if last:
                    nc.scalar.activation(outT[0:D, hp, cs], pvM[0:D, 0:512],
                                         Copy)
                    nc.scalar.activation(outT[D:P, hp, cs], pvM[0:D, 512:1024],
                                         Copy)
                else:
                    nc.vector.tensor_copy(outT[0:D, hp, cs], pvM[0:D, 0:512])
                    nc.vector.tensor_copy(outT[D:P, hp, cs],
                                          pvM[0:D, 512:1024])
                nc.vector.tensor_tensor(
                    outT[0:D, hp, cs], outT[0:D, hp, cs], rec[0:D, :], mult
                )
                nc.vector.tensor_tensor(
                    outT[D:P, hp, cs], outT[D:P, hp, cs], rec[D:P, :], mult
                )
            return drain

        opens = {}

        def o_tile(m):
            def emit():
                py = ps.tile([P, 512], dt, tag="w", name=f"py{m}")
                for et in range(KT4):
                    nc.tensor.matmul(
                        py[:],
                        lhsT=outT[:, et, m * P:(m + 1) * P],
                        rhs=wo_s[:, et, :],
                        start=et == 0,
                        stop=et == KT4 - 1,
                    )
                yt = yp.tile([P, 512], dt, tag="y", name=f"yt{m}")
                nc.vector.tensor_tensor(yt[:], py[:], bor_s[:], add)
                nc.sync.dma_start(y_d[m * P:(m + 1) * P, :], yt[:])
            return emit

        def o_pre(m):
            # accumulate the first 3 contraction tiles of output chunk m,
            # leaving the PSUM group open for the last tile post-drain
            def emit():
                py = ps.tile([P, 512], dt, tag="w", name=f"py{m}")
                opens[m] = py
                for et in range(3):
                    nc.tensor.matmul(
                        py[:],
                        lhsT=outT[:, et, m * P:(m + 1) * P],
                        rhs=wo_s[:, et, :],
                        start=et == 0,
                        stop=False,
                    )
            return emit

        def o_fin(m):
            py = opens.pop(m)
            nc.tensor.matmul(
                py[:],
                lhsT=outT[:, 3, m * P:(m + 1) * P],
                rhs=wo_s[:, 3, :],
                start=False,
                stop=True,
            )
            yt = yp.tile([P, 512], dt, tag="y", name=f"yt{m}")
            nc.vector.tensor_tensor(yt[:], py[:], bor_s[:], add)
            nc.sync.dma_start(y_d[m * P:(m + 1) * P, :], yt[:])

        # Prerequisites for block (1,0) that can't ride a fill slot: Q(1,c=0)
        # and K(1,c=0), emitted right after the startup drains (all xt/weights
        # are resident by then; these overlap block (0,0)'s exp stream).
        q_tile(1, 0)()
        k_tile(1, 0)()

        # chunk-outer: chunk 0's output projection + DMA overlap chunk 1's
        # ScalarE-bound attention.  Projections and chunk-0 output tiles are
        # placed just-in-time in later blocks' t-slots (the PE has slack while
        # ScalarE works through the exp stream), and each block's drain is
        # deferred into the next block so its recip chain never head-of-line-
        # blocks the score matmuls.  Block (0,0) carries the 16 V-projection
        # tiles and no other fills.
        fills = {
            (1, 0): {1: k_tile(1, 1), 3: k_tile(1, 2), 5: k_tile(1, 3),
                     7: q_tile(2, 0), 9: k_tile(2, 0)},
            (2, 0): {1: k_tile(2, 1), 3: k_tile(2, 2), 5: k_tile(2, 3),
                     7: q_tile(3, 0), 9: k_tile(3, 0)},
            (3, 0): {1: k_tile(3, 1), 3: k_tile(3, 2), 5: k_tile(3, 3),
                     7: q_tile(1, 1)},
            (0, 1): {5: q_tile(2, 1)},
            (1, 1): {3: o_tile(0), 5: q_tile(3, 1), 8: o_tile(1),
                     11: o_tile(2)},
            (2, 1): {3: o_tile(3)},
            (3, 1): {8: o_pre(4), 11: o_pre(5)},
        }
        drain = None
        for c in range(2):
            for hp in range(4):
                drain = emit_attn(hp, c, with_v=(c == 0 and hp == 0),
                                  fill=fills.get((hp, c), {}),
                                  drain_prev=drain)
        drain(last=True)
        o_fin(4)
        o_fin(5)
        for m in range(6, 8):
            o_tile(m)()

        if debug:
            nc.sync.dma_start(dQT.ap(), QT[:])
            nc.sync.dma_start(dKT.ap(), KTt[:])
            nc.sync.dma_start(dVb.ap(), Vb[:])
            nc.sync.dma_start(doutT.ap(), outT[:])

    nc.finalize()
    return nc


def _get_program(debug=False):
    key = ("nc", debug)
    if key not in _CACHE:
        _CACHE[key] = _build_program(debug)
    return _CACHE[key]


def _host_inputs(x, Wq, bq, Wk, bk, Wv, bv, Wo, bo):
    import ml_dtypes
    f32 = np.float32
    bf = ml_dtypes.bfloat16
    wqT = np.ascontiguousarray(np.asarray(Wq, f32).T).reshape(KT4, P, EMB).astype(bf)
    wkT = np.ascontiguousarray(np.asarray(Wk, f32).T).reshape(KT4, P, EMB).astype(bf)
    wvT = np.ascontiguousarray(np.asarray(Wv, f32).T).reshape(KT4, P, EMB).astype(bf)
    woT = np.ascontiguousarray(np.asarray(Wo, f32).T).reshape(KT4, P, EMB).astype(bf)
    bq2 = np.ascontiguousarray(np.asarray(bq, f32).reshape(KT4, P).T)
    bk2 = np.ascontiguousarray(np.asarray(bk, f32).reshape(KT4, P).T)
    bvr = np.ascontiguousarray(np.tile(np.asarray(bv, f32), (P, 1)))
    bor = np.ascontiguousarray(np.tile(np.asarray(bo, f32), (P, 1)))
    sel2 = np.zeros((2, P), f32)
    for m in range(P):
        sel2[m // D, m] = 1.0
    sel2 = sel2.astype(bf)

    shared = dict(wq=wqT, wk=wkT, wv=wvT, wo=woT, bq2=bq2, bk2=bk2,
                  bvr=bvr, bor=bor, sel2=sel2)
    x = np.asarray(x, f32)
    in_maps = []
    for c in range(NCORES):
        b, hf = c // 2, c % 2
        xb = x[b]
        # queries first; key order is irrelevant as long as K and V agree
        xr = np.concatenate(
            [xb[hf * HALF:(hf + 1) * HALF], xb[(1 - hf) * HALF:(2 - hf) * HALF]], 0
        )
        xT = np.ascontiguousarray(xr.T).reshape(KT4, P, N).astype(bf)
        in_maps.append(dict(shared, xT=xT))
    return in_maps


def kernel(x, Wq, bq, Wk, bk, Wv, bv, Wo, bo, _trace=False, _trace_cores=None,
           _debug=False):
    from concourse.bass_utils import run_bass_kernel_spmd

    nc = _get_program(_debug)
    in_maps = _host_inputs(x, Wq, bq, Wk, bk, Wv, bv, Wo, bo)
    res = run_bass_kernel_spmd(
        nc, in_maps, list(range(NCORES)), trace=_trace,
        trace_cores=_trace_cores,
    )
    out = np.empty((B, N, EMB), np.float32)
    for c in range(NCORES):
        b, hf = c // 2, c % 2
        out[b, hf * HALF:(hf + 1) * HALF] = res.results[c]["y"]
    if _trace:
        _CACHE["last_results"] = res
    return out
